# revision 1
# baseline (speedup 1.0000x reference)
"""Trainium2 Bass kernel for nn_BiLSTM: 2-layer BiLSTM (B=64,T=512,D=64,H=128) + FC.

Sharding: data-parallel over batch across 8 NeuronCores (8 samples/core).
Raw bass (no Tile) with manual semaphores; single compute stream per engine.

Per-core dataflow:
  x [8,T,64] --DMA--> x_stage [128,4T] --PE transpose--> X0 [64, T*8] (d, t*8+b) bf16
  layer l, dir d: gate pre-acts accumulate in PSUM gate-major:
      bank[:, g*128 + r*16 + dir*8 + b]   (r = slot region)
  bias via mask-matmul, pregate via X@Wih (chunked 8 slots), recurrence via
  W-stationary matmuls [128gu, 8b].  sigma-everywhere: one Sigmoid over all 4
  gate blocks (g pre-acts pre-scaled x2 on host), tanh(g)=2*sig(2x)-1 via DVE.
  fwd step s pairs with bwd step s-1 (one-slot stagger, shared sigma op).
  h outputs -> X1f/X1b -> layer 1 -> X2f/X2b -> FC (+bias row) -> y [64, T*8].
Host: reshape y -> [8,T,64] per core, concat cores -> [64,T,64].
"""
import sys, os
sys.path.insert(0, "/opt/trn_rl_repo")
import numpy as np
import ml_dtypes

import concourse.bass as bass
from concourse import mybir
from concourse.bass_utils import run_bass_kernel_spmd

F32 = mybir.dt.float32
BF16 = mybir.dt.bfloat16
BF = ml_dtypes.bfloat16
AluOp = mybir.AluOpType
ActFn = mybir.ActivationFunctionType

H = 128
NB = 4  # rotating PSUM gate banks
BLK = {"i": 0, "f": 1, "o": 2, "g": 3}          # PSUM gate-block order
PT = {"i": 0, "f": 1, "g": 2, "o": 3}           # PyTorch row-block order


def ap_of(t, off, dims):
    base = t[:] if not isinstance(t, bass.AP) else t
    return bass.AP(tensor=base.tensor, offset=base.offset + off, ap=list(dims))


def pstride(t):
    base = t[:] if not isinstance(t, bass.AP) else t
    return base.ap[0][0]


def build_nc(T=512, dbg=False, serial=False, nosync=False):
    assert T % 16 == 0
    NTOK = T * 8
    nc = bass.Bass("TRN2", target_bir_lowering=False, debug=False)
    dbg_d = {}
    if dbg:
        for nm, shp in [("dX0", [64, NTOK]), ("dX1f", [128, NTOK]),
                        ("dX1b", [128, NTOK]), ("dX2f", [128, NTOK]),
                        ("dX2b", [128, NTOK]), ("dU", [128, 64]),
                        ("dU2", [128, 64]), ("dCF0", [128, 8]), ("dVF0", [128, 8]),
                        ("dUS0", [128, 64]), ("dPF0", [128, 8]), ("dQF0", [128, 8])]:
            dbg_d[nm] = nc.dram_tensor(nm, shp, F32, kind="ExternalOutput")

    # ---------------- DRAM I/O ----------------
    x_d = nc.dram_tensor("x", [8, T, 64], F32, kind="ExternalInput")
    wih0 = {d: nc.dram_tensor(f"wih0{d}", [64, 512], BF16, kind="ExternalInput") for d in "fb"}
    wih1 = {d: nc.dram_tensor(f"wih1{d}", [256, 512], BF16, kind="ExternalInput") for d in "fb"}
    whh_d = {(l, d): nc.dram_tensor(f"whh{l}{d}", [128, 512], BF16, kind="ExternalInput")
             for l in (0, 1) for d in "fb"}
    bias8_d = {l: nc.dram_tensor(f"bias8_{l}", [8, 128], BF16, kind="ExternalInput")
               for l in (0, 1)}
    wfc_d = nc.dram_tensor("wfc", [256, 64], BF16, kind="ExternalInput")
    mask8_d = nc.dram_tensor("mask8_in", [8, 512], BF16, kind="ExternalInput")
    id128_d = nc.dram_tensor("id128_in", [128, 128], F32, kind="ExternalInput")
    ones_d = nc.dram_tensor("ones_in", [1, 512], F32, kind="ExternalInput")
    zero8_d = nc.dram_tensor("zero8_in", [128, 8], BF16, kind="ExternalInput")
    bfc_d = nc.dram_tensor("bfc", [1, 64], F32, kind="ExternalInput")
    y_d = nc.dram_tensor("y", [64, NTOK], F32, kind="ExternalOutput")

    # ---------------- SBUF ----------------
    sb = nc.alloc_sbuf_tensor
    x_stage = sb("x_stage", [128, 4 * T], F32)
    X0 = sb("X0", [64, NTOK], BF16)
    Xf = {1: sb("X1f", [128, NTOK], BF16), 2: sb("X2f", [128, NTOK], BF16)}
    Xb = {1: sb("X1b", [128, NTOK], BF16), 2: sb("X2b", [128, NTOK], BF16)}
    y_s = sb("y_s", [64, NTOK], F32)

    w_ih0 = {d: sb(f"w_ih0{d}", [64, 512], BF16) for d in "fb"}
    w_ih1a = {d: sb(f"w_ih1a{d}", [128, 512], BF16) for d in "fb"}
    w_ih1b = {d: sb(f"w_ih1b{d}", [128, 512], BF16) for d in "fb"}
    w_hh = {(l, d): sb(f"w_hh{l}{d}", [128, 512], BF16) for l in (0, 1) for d in "fb"}
    b8 = {l: sb(f"b8_{l}", [8, 128], BF16) for l in (0, 1)}
    wfca = sb("wfca", [128, 64], BF16)
    wfcb = sb("wfcb", [128, 64], BF16)
    bfc = sb("bfc_s", [1, 64], F32)

    mask8 = sb("mask8", [8, 512], BF16)
    ones_fc = sb("ones_fc", [1, 512], F32)
    id128 = sb("id128", [128, 128], F32)
    zero8 = sb("zero8", [128, 8], BF16)
    u_t = [sb(f"u{i}", [128, 64], BF16) for i in range(2)]
    dbg_snap = {"dCF0": sb("s_dCF0", [128, 8], F32),
                "dVF0": sb("s_dVF0", [128, 8], F32),
                "dUS0": sb("s_dUS0", [128, 64], F32),
                "dPF0": sb("s_dPF0", [128, 8], F32),
                "dQF0": sb("s_dQF0", [128, 8], F32)} if dbg else None
    c_t = {d: sb(f"c_{d}", [128, 8], F32) for d in "fb"}
    p_t = {d: sb(f"p_{d}", [128, 8], BF16) for d in "fb"}
    q_t = {d: sb(f"q_{d}", [128, 8], F32) for d in "fb"}
    v_t = {d: sb(f"v_{d}", [128, 8], BF16) for d in "fb"}
    spacer8 = sb("spacer8", [128, 8], F32)

    gbank = [nc.alloc_psum_tensor(f"gb{i}", [128, 512], F32) for i in range(NB)]
    tbank = [nc.alloc_psum_tensor(f"tb{i}", [64, 512], F32) for i in range(2)]

    sem_in = nc.alloc_semaphore("sem_in")
    s_mm = nc.alloc_semaphore("s_mm")
    s_act = nc.alloc_semaphore("s_act")
    s_dve = nc.alloc_semaphore("s_dve")
    s_out = nc.alloc_semaphore("s_out")
    cnt = {"mm": 0, "act": 0, "dve": 0}

    sems = {"mm": s_mm, "act": s_act, "dve": s_dve}

    def W(eng, sem, val):
        if not nosync:
            eng.wait_ge(sem, val)

    def inc(ins, which, sem):
        ins.then_inc(sem, 1)
        cnt[which] += 1
        if serial:
            for eng in (nc.tensor, nc.scalar, nc.vector):
                for w in ("mm", "act", "dve"):
                    eng.wait_ge(sems[w], cnt[w])
        return cnt[which]

    # ---------------- input DMAs (sync engine queues) ----------------
    n_dma = 0

    def dma(dst, src):
        nonlocal n_dma
        nc.sync.dma_start(out=dst, in_=src).then_inc(sem_in, 16)
        n_dma += 1

    dma(x_stage[:, :], x_d[:].rearrange("b t d -> (b t d)").rearrange("(p f) -> p f", p=128))
    for d in "fb":
        dma(w_ih0[d][:, :], wih0[d][:, :])
        dma(w_ih1a[d][:, :], wih1[d][0:128, :])
        dma(w_ih1b[d][:, :], wih1[d][128:256, :])
        dma(w_hh[(0, d)][:, :], whh_d[(0, d)][:, :])
        dma(w_hh[(1, d)][:, :], whh_d[(1, d)][:, :])
    for l in (0, 1):
        dma(b8[l][:, :], bias8_d[l][:, :])
    dma(wfca[:, :], wfc_d[0:128, :])
    dma(wfcb[:, :], wfc_d[128:256, :])
    dma(bfc[:, :], bfc_d[:, :])
    dma(mask8[:, :], mask8_d[:, :])
    dma(id128[:, :], id128_d[:, :])
    dma(ones_fc[:, :], ones_d[:, :])
    dma(zero8[:, :], zero8_d[:, :])

    nc.tensor.wait_ge(sem_in, 16 * n_dma)


    # ---------------- x transpose into X0 ----------------
    TL = T // 16          # t_low values per partition-row
    copy_done = {}        # tlo -> ("act"/"dve", count)
    for tlo in range(TL):
        bank = tbank[tlo % 2]
        if tlo >= 2:
            eng, c0 = copy_done[tlo - 2]
            W(nc.tensor, s_act if eng == "act" else s_dve, c0)
        ins = nc.tensor.transpose(bank[0:64, 0:128],
                                  x_stage[:, tlo * 64:(tlo + 1) * 64], id128[:, :])
        trc = inc(ins, "mm", s_mm)
        src = ap_of(bank, 0, [[pstride(bank), 64], [1, 16], [16, 8]])
        dst = ap_of(X0, tlo * 8, [[pstride(X0), 64], [TL * 8, 16], [1, 8]])
        if tlo % 4 < 2:
            W(nc.scalar, s_mm, trc)
            ins = nc.scalar.activation(dst, src, ActFn.Copy)
            copy_done[tlo] = ("act", inc(ins, "act", s_act))
        else:
            W(nc.vector, s_mm, trc)
            ins = nc.vector.tensor_copy(dst, src)
            copy_done[tlo] = ("dve", inc(ins, "dve", s_dve))

    # ---------------- BiLSTM layers ----------------
    def gates_ap(bank, g, r, dd, nb=8, nr=1):
        off = g * 128 + r * 16 + dd * 8
        dims = [[pstride(bank), 128]]
        if nr > 1:
            dims.append([16, nr])
        dims.append([1, nb])
        return ap_of(bank, off, dims)

    def layer(l, parts_f, parts_b, XfO, XbO):
        """parts_*: list of (lhsT_sbuf, src_ap_tensor, Krows) for that direction."""
        n_chunks = T // 8 + 1
        sig_done, hf_done, hb_done, cpf, cpb = {}, {}, {}, {}, {}

        # barrier: everything ACT/DVE emitted so far must be done before PE
        # writes gate banks / reads X sources of this layer
        W(nc.tensor, s_act, cnt["act"])
        W(nc.tensor, s_dve, cnt["dve"])
        nc.vector.memset(c_t["f"][:, :], 0.0)
        nc.vector.memset(c_t["b"][:, :], 0.0)

        def pregate(c):
            if c >= n_chunks:
                return
            bank = gbank[c % NB]
            nc.tensor.matmul(bank[:, :], b8[l][:, :], mask8[:, :],
                             start=True, stop=False, skip_group_check=True)
            t0, t1 = 8 * c, min(8 * c + 8, T)
            if t0 < t1:
                for (lhsT, src, kr) in parts_f:
                    for g in range(4):
                        nc.tensor.matmul(
                            gates_ap(bank, g, t0 % 8, 0, nb=8, nr=t1 - t0),
                            lhsT[0:kr, g * 128:(g + 1) * 128],
                            src[0:kr, t0 * 8:t1 * 8],
                            start=False, stop=False, skip_group_check=True)
            # bwd: region rho holds step j = 8c+6-rho at time tt = (T-7)-8c+rho
            rhos = [rho for rho in range(8)
                    if 0 <= (T - 7) - 8 * c + rho <= T - 1 and 0 <= 8 * c + 6 - rho <= T - 1]
            if rhos:
                r0, r1 = rhos[0], rhos[-1]
                tt0 = (T - 7) - 8 * c + r0
                nr = r1 - r0 + 1
                for (lhsT, src, kr) in parts_b:
                    for g in range(4):
                        dst = ap_of(bank, g * 128 + r0 * 16 + 8,
                                    [[pstride(bank), 128], [16, nr], [1, 8]])
                        nc.tensor.matmul(dst, lhsT[0:kr, g * 128:(g + 1) * 128],
                                         src[0:kr, tt0 * 8:(tt0 + nr) * 8],
                                         start=False, stop=False, skip_group_check=True)

        for c in range(min(NB, n_chunks)):
            pregate(c)

        for s in range(T + 1):
            bank = gbank[(s // 8) % NB]
            r = s % 8
            # PE: bwd rec MMs for step j=s-1  (region 7-r, cols +8)
            if s >= 1:
                j = s - 1
                if j >= 1:
                    W(nc.tensor, s_dve, hb_done[j - 1])
                rhs = zero8[:, :] if j == 0 else XbO[:, (T - j) * 8:(T - j + 1) * 8]
                for g in range(4):
                    nc.tensor.matmul(gates_ap(bank, g, 7 - r, 1),
                                     w_hh[(l, "b")][:, g * 128:(g + 1) * 128],
                                     rhs, start=False, stop=True, skip_group_check=True)
            # PE: fwd rec MMs for step s  (region r, cols +0)
            if s <= T - 1:
                if s >= 1:
                    W(nc.tensor, s_dve, hf_done[s - 1])
                rhs = zero8[:, :] if s == 0 else XfO[:, (s - 1) * 8:s * 8]
                last = None
                for g in range(4):
                    last = nc.tensor.matmul(gates_ap(bank, g, r, 0),
                                            w_hh[(l, "f")][:, g * 128:(g + 1) * 128],
                                            rhs, start=False, stop=True,
                                            skip_group_check=True)
                mm_here = inc(last, "mm", s_mm)
            else:
                mm_here = inc(nc.tensor.nop(), "mm", s_mm)

            # ACT: sigma over [4 gate blocks] x [fwd block, bwd block] x [8]
            a_f, a_b = r * 16, (7 - r) * 16 + 8
            first = min(a_f, a_b)
            delta = abs(a_b - a_f)
            off_f = 0 if a_f < a_b else 8
            off_b = 8 - off_f
            src = ap_of(bank, first, [[pstride(bank), 128], [128, 4], [delta, 2], [1, 8]])
            u = u_t[s % 2]
            dst = ap_of(u, 0, [[pstride(u), 128], [16, 4], [8, 2], [1, 8]])
            W(nc.scalar, s_mm, mm_here)
            ins = nc.scalar.activation(dst, src, ActFn.Sigmoid)
            sig_done[s] = inc(ins, "act", s_act)

            # PE: pregate burst for chunk c+NB into the bank just freed
            if r == 7:
                cc = s // 8 + NB
                if cc < n_chunks:
                    W(nc.tensor, s_act, sig_done[s])
                    pregate(cc)

            # DVE cells: c_tilde = c/2:  c~' = (u_g-0.5)*u_i + u_f*c~ ; tanh scale=2
            # NOTE: DVE gap-0 RAW hazard -- a DVE op must not read the output of
            # the immediately preceding DVE op.  Interleave dirs to guarantee gap>=1.
            def u_blk(gname, off):
                o0 = BLK[gname] * 16 + off
                return u[:, o0:o0 + 8]

            dirs = []
            if s <= T - 1:
                dirs.append(("f", off_f))
            if s >= 1:
                dirs.append(("b", off_b))
            W(nc.vector, s_act, sig_done[s])
            for dd, off in dirs:
                nc.vector.scalar_tensor_tensor(out=p_t[dd][:, :], in0=u_blk("g", off),
                                               scalar=0.5, in1=u_blk("i", off),
                                               op0=AluOp.subtract, op1=AluOp.mult)
            for dd, off in dirs:
                nc.vector.tensor_tensor(out=q_t[dd][:, :], in0=u_blk("f", off),
                                        in1=c_t[dd][:, :], op=AluOp.mult)
            if len(dirs) == 1:
                nc.vector.memset(spacer8[:, :], 0.0)   # break gap-0 q->c' pair
            for dd, off in dirs:
                ins = nc.vector.tensor_tensor(out=c_t[dd][:, :], in0=p_t[dd][:, :],
                                              in1=q_t[dd][:, :], op=AluOp.add)
                which = inc(ins, "dve", s_dve)
                if dd == "f":
                    cpf[s] = which
                else:
                    cpb[s - 1] = which

            if s <= T - 1:
                W(nc.scalar, s_dve, cpf[s])
                ins = nc.scalar.activation(v_t["f"][:, :], c_t["f"][:, :], ActFn.Tanh,
                                           scale=2.0)
                tf = inc(ins, "act", s_act)
                W(nc.vector, s_act, tf)
                ins = nc.vector.tensor_tensor(
                    out=XfO[:, s * 8:(s + 1) * 8],
                    in0=u[:, BLK["o"] * 16 + off_f: BLK["o"] * 16 + off_f + 8],
                    in1=v_t["f"][:, :], op=AluOp.mult)
                hf_done[s] = inc(ins, "dve", s_dve)
                if dbg and l == 1 and s == 0:
                    nc.vector.tensor_copy(dbg_snap["dPF0"][:, :], p_t["f"][:, :])
                    nc.vector.tensor_copy(dbg_snap["dQF0"][:, :], q_t["f"][:, :])
                    nc.vector.tensor_copy(dbg_snap["dCF0"][:, :], c_t["f"][:, :])
                    nc.vector.tensor_copy(dbg_snap["dVF0"][:, :], v_t["f"][:, :])
                    nc.vector.tensor_copy(dbg_snap["dUS0"][:, :], u[:, :])
            if s >= 1:
                j = s - 1
                W(nc.scalar, s_dve, cpb[j])
                ins = nc.scalar.activation(v_t["b"][:, :], c_t["b"][:, :], ActFn.Tanh,
                                           scale=2.0)
                tb = inc(ins, "act", s_act)
                W(nc.vector, s_act, tb)
                ins = nc.vector.tensor_tensor(
                    out=XbO[:, (T - 1 - j) * 8:(T - j) * 8],
                    in0=u[:, BLK["o"] * 16 + off_b: BLK["o"] * 16 + off_b + 8],
                    in1=v_t["b"][:, :], op=AluOp.mult)
                hb_done[j] = inc(ins, "dve", s_dve)

    layer(0, [(w_ih0["f"], X0, 64)], [(w_ih0["b"], X0, 64)], Xf[1], Xb[1])
    layer(1, [(w_ih1a["f"], Xf[1], 128), (w_ih1b["f"], Xb[1], 128)],
          [(w_ih1a["b"], Xf[1], 128), (w_ih1b["b"], Xb[1], 128)], Xf[2], Xb[2])

    # ---------------- FC ----------------
    W(nc.tensor, s_act, cnt["act"])
    W(nc.tensor, s_dve, cnt["dve"])
    fc_copy = {}
    fc_starts = list(range(0, NTOK, 512))
    for i, st in enumerate(fc_starts):
        w = min(512, NTOK - st)
        bank = tbank[i % 2]
        if i >= 2:
            eng, c0 = fc_copy[i - 2]
            W(nc.tensor, s_act if eng == "act" else s_dve, c0)
        nc.tensor.matmul(bank[0:64, 0:w], bfc[:, :], ones_fc[:, 0:w],
                         start=True, stop=False, skip_group_check=True)
        nc.tensor.matmul(bank[0:64, 0:w], wfca[:, :], Xf[2][:, st:st + w],
                         start=False, stop=False, skip_group_check=True)
        ins = nc.tensor.matmul(bank[0:64, 0:w], wfcb[:, :], Xb[2][:, st:st + w],
                               start=False, stop=True, skip_group_check=True)
        mmc = inc(ins, "mm", s_mm)
        if i % 2 == 0:
            W(nc.scalar, s_mm, mmc)
            ins = nc.scalar.activation(y_s[:, st:st + w], bank[0:64, 0:w],
                                       ActFn.Copy)
            fc_copy[i] = ("act", inc(ins, "act", s_act))
        else:
            W(nc.vector, s_mm, mmc)
            ins = nc.vector.tensor_copy(y_s[:, st:st + w], bank[0:64, 0:w])
            fc_copy[i] = ("dve", inc(ins, "dve", s_dve))

    # ---------------- output DMA ----------------
    nc.sync.wait_ge(s_act, cnt["act"])
    nc.sync.wait_ge(s_dve, cnt["dve"])
    n_out = 0
    def dma_out(dst, src):
        nonlocal n_out
        nc.sync.dma_start(out=dst, in_=src).then_inc(s_out, 16)
        n_out += 1
    dma_out(y_d[:, :], y_s[:, :])
    if dbg:
        # cast debug bf16 buffers to f32 via DVE into y-staging-like temps
        dcast = sb("dcast", [128, max(NTOK, 512)], F32)
        for nm, buf, npart in [("dX0", X0, 64), ("dX1f", Xf[1], 128), ("dX1b", Xb[1], 128),
                               ("dX2f", Xf[2], 128), ("dX2b", Xb[2], 128)]:
            nc.vector.wait_ge(s_out, 16 * n_out)   # prior DMA from dcast done
            ins = nc.vector.tensor_copy(dcast[0:npart, 0:NTOK], buf[:, :])
            cc = inc(ins, "dve", s_dve)
            nc.sync.wait_ge(s_dve, cc)
            dma_out(dbg_d[nm][0:npart, :], dcast[0:npart, 0:NTOK])
        for bi in range(NB):
            dbg_d[f"dGB{bi}"] = nc.dram_tensor(f"dGB{bi}", [128, 512], F32,
                                               kind="ExternalOutput")
            nc.vector.wait_ge(s_out, 16 * n_out)
            ins = nc.vector.tensor_copy(dcast[:, 0:512], gbank[bi][:, :])
            cc = inc(ins, "dve", s_dve)
            nc.sync.wait_ge(s_dve, cc)
            dma_out(dbg_d[f"dGB{bi}"][:, :], dcast[:, 0:512])
        ucast = sb("ucast", [128, 64], F32)
        ins = nc.vector.tensor_copy(ucast[:, :], u_t[(T) % 2][:, :])
        cc = inc(ins, "dve", s_dve)
        nc.sync.wait_ge(s_dve, cc)
        dma_out(dbg_d["dU"][:, :], ucast[:, :])
        for snm, sbuf_t in dbg_snap.items():
            npp, nff = sbuf_t[:].ap[0][1], sbuf_t[:].ap[-1][1]
            nc.sync.wait_ge(s_dve, cnt["dve"])
            dma_out(dbg_d[snm][:, :], sbuf_t[:, :])
        ucast2 = sb("ucast2", [128, 64], F32)
        ins = nc.vector.tensor_copy(ucast2[:, :], u_t[(T + 1) % 2][:, :])
        cc = inc(ins, "dve", s_dve)
        nc.sync.wait_ge(s_dve, cc)
        dma_out(dbg_d["dU2"][:, :], ucast2[:, :])
    nc.sync.wait_ge(s_out, 16 * n_out)
    return nc


# ====================== host-side prep & entry point ======================

def _to_bf(a):
    return np.asarray(a, dtype=np.float32).astype(BF)


def prep_weights(inp, l, suf_f, suf_b):
    """Build per-layer lhsT tensors + bias8 from PyTorch-layout weights."""
    out = {}
    for dname, suf in (("f", suf_f), ("b", suf_b)):
        wih = np.asarray(inp[f"w_ih_l{l}{suf}"], np.float32)   # [512, Din]
        whh = np.asarray(inp[f"w_hh_l{l}{suf}"], np.float32)   # [512, 128]
        bsum = (np.asarray(inp[f"b_ih_l{l}{suf}"], np.float32)
                + np.asarray(inp[f"b_hh_l{l}{suf}"], np.float32))  # [512]
        blocks_ih, blocks_hh, bias_rows = [], [], {}
        for gname, blk in BLK.items():
            rows = slice(PT[gname] * 128, (PT[gname] + 1) * 128)
            scale = 2.0 if gname == "g" else 1.0
            blocks_ih.append((scale * wih[rows]).T)            # [Din, 128]
            blocks_hh.append((scale * whh[rows]).T)            # [128, 128]
            bias_rows[blk] = scale * bsum[rows]
        out[f"wih_{dname}"] = _to_bf(np.concatenate(blocks_ih, axis=1))  # [Din, 512]
        out[f"whh_{dname}"] = _to_bf(np.concatenate(blocks_hh, axis=1))  # [128, 512]
        out[f"bias_{dname}"] = bias_rows
    bias8 = np.zeros((8, 128), np.float32)
    for blk in range(4):
        bias8[blk * 2 + 0] = out["bias_f"][blk]
        bias8[blk * 2 + 1] = out["bias_b"][blk]
    out["bias8"] = _to_bf(bias8)
    return out


def _mask8_np():
    m = np.zeros((8, 512), np.float32)
    for j in range(8):
        g, dd = j // 2, j % 2
        for r in range(8):
            m[j, g * 128 + r * 16 + dd * 8: g * 128 + r * 16 + dd * 8 + 8] = 1.0
    return m.astype(BF)


_NC_CACHE = {}


def _get_nc(T, dbg=False, serial=False):
    key = (T, dbg, serial)
    if key not in _NC_CACHE:
        _NC_CACHE[key] = build_nc(T, dbg, serial)
    return _NC_CACHE[key]


def run_cores(inputs, T=512, n_cores=8, trace=False, dbg=False, serial=False):
    x = np.asarray(inputs["x"], np.float32)
    per = 8

    l0 = prep_weights(inputs, 0, "", "r")
    l1 = prep_weights(inputs, 1, "", "r")
    wfc = _to_bf(np.asarray(inputs["w_fc"], np.float32).T)       # [256, 64]
    bfc = np.asarray(inputs["b_fc"], np.float32).reshape(1, 64)

    common = {
        "wih0f": l0["wih_f"], "wih0b": l0["wih_b"],
        "wih1f": l1["wih_f"], "wih1b": l1["wih_b"],
        "whh0f": l0["whh_f"], "whh0b": l0["whh_b"],
        "whh1f": l1["whh_f"], "whh1b": l1["whh_b"],
        "bias8_0": l0["bias8"], "bias8_1": l1["bias8"],
        "wfc": wfc, "bfc": bfc,
        "mask8_in": _mask8_np(), "id128_in": np.eye(128, dtype=np.float32),
        "ones_in": np.ones((1, 512), np.float32),
        "zero8_in": np.zeros((128, 8), np.float32).astype(BF),
    }
    in_maps = []
    for c in range(n_cores):
        m = dict(common)
        m["x"] = np.ascontiguousarray(x[c * per:(c + 1) * per, :T])
        in_maps.append(m)

    nc = _get_nc(T, dbg, serial)
    res = run_bass_kernel_spmd(nc, in_maps, core_ids=list(range(n_cores)),
                               trace=trace)
    outs = []
    for c in range(n_cores):
        yc = res.results[c]["y"]                      # [64, T*8]
        outs.append(yc.reshape(64, T, 8).transpose(2, 1, 0))
    return np.concatenate(outs, axis=0), res


def kernel(**inputs):
    y, _ = run_cores(inputs, T=512, n_cores=8)
    return y.astype(np.float32)



# revision 2
# speedup vs baseline: 1118.4247x; 1118.4247x over previous
"""Trainium2 Bass kernel for nn_BiLSTM: 2-layer BiLSTM (B=64,T=512,D=64,H=128) + FC.

Sharding: data-parallel over batch across 8 NeuronCores (8 samples/core).

v2 design (split-dir, in-phase lockstep, min-chain cell update):
  Per layer, fwd and bwd run as two independent recurrent chains advanced in
  lockstep; per step s each dir does:
    PE : 4 gate matmuls  gates += Whh_g @ h~(s-1)    [128,8] into PSUM bank
    ACT: sg = Sigmoid(bank[32 cols])          -> u[par][0:32]   (i,f,g,o x 8)
    DVE: PQ = (in0 - .5) * in1  where in0=[g~|C], in1=[i~|f~]   -> [p|q]
    DVE: C' = (p + .5) + q                    -> u[par^1][32:40]
    ACT: v^ = Sigmoid(4C' - 2)                -> vhat
    DVE: h~ = (v^ - .5) * o~                  -> X[l] column (bf16)
  State: C = c/2 + 0.5 kept adjacent to the sigma outputs so PQ is one
  strided-AP op.  h~ = h/2; consumers (Whh, Wih_l1, Wfc) pre-scaled by 2.
  g-gate rows pre-scaled by 2 so one Sigmoid covers tanh(g) via 2sig(2g)-1.

  Engine queue order per step: PE [mmf x4, mmb x4, pregate burst]
                               ACT [sgf, sgb, scf, scb]
                               DVE [PQf, PQb, Cf, Cb, hf, hb]
  Pregates (x-part + bias) accumulate into 2 PSUM banks/dir (16 steps each),
  staged 2 chunks ahead during PE idle; no explicit waits needed (transitively
  ordered through the h-dependency).
Host: reshape y -> [8,T,64] per core, concat cores -> [64,T,64].
"""
import sys
sys.path.insert(0, "/opt/trn_rl_repo")
import numpy as np
import ml_dtypes

import concourse.bass as bass
from concourse import mybir
from concourse.bass_utils import run_bass_kernel_spmd

F32 = mybir.dt.float32
BF16 = mybir.dt.bfloat16
BF = ml_dtypes.bfloat16
AluOp = mybir.AluOpType
ActFn = mybir.ActivationFunctionType

H = 128
PT = {"i": 0, "f": 1, "g": 2, "o": 3}   # PyTorch row-block order
GO = ["i", "f", "g", "o"]               # PSUM/u col-block order (8 cols each)
DIRS = ("f", "b")


def ap_of(t, off, dims):
    base = t[:] if not isinstance(t, bass.AP) else t
    return bass.AP(tensor=base.tensor, offset=base.offset + off, ap=list(dims))


def pstride(t):
    base = t[:] if not isinstance(t, bass.AP) else t
    return base.ap[0][0]


def build_nc(T=512):
    assert T % 16 == 0
    NTOK = T * 8
    NCH = T // 16                      # pregate chunks (16 steps each)
    nc = bass.Bass("TRN2", target_bir_lowering=False, debug=False)

    # register -2.0 const AP (sigma-cell bias), same pattern as Bass.__init__
    _c = nc.alloc_sbuf_tensor("const-f32-neg2", [128, 1], F32)
    nc.gpsimd.memset(_c.ap(), -2.0)
    nc.const_aps.aps[(F32, -2.0)] = _c.ap()
    nc.all_engine_barrier()

    # ---------------- DRAM I/O ----------------
    x_d = nc.dram_tensor("x", [8, T, 64], F32, kind="ExternalInput")
    whh_d = {(l, d): nc.dram_tensor(f"whh{l}{d}", [128, 512], BF16, kind="ExternalInput")
             for l in (0, 1) for d in DIRS}
    wih0_d = {d: nc.dram_tensor(f"wih0{d}", [64, 512], BF16, kind="ExternalInput") for d in DIRS}
    wih1a_d = {d: nc.dram_tensor(f"wih1a{d}", [128, 512], BF16, kind="ExternalInput") for d in DIRS}
    wih1b_d = {d: nc.dram_tensor(f"wih1b{d}", [128, 512], BF16, kind="ExternalInput") for d in DIRS}
    b4_d = {(l, d): nc.dram_tensor(f"b4_{l}{d}", [4, 128], BF16, kind="ExternalInput")
            for l in (0, 1) for d in DIRS}
    mask4_d = nc.dram_tensor("mask4_in", [4, 512], BF16, kind="ExternalInput")
    wfca_d = nc.dram_tensor("wfca", [128, 64], BF16, kind="ExternalInput")
    wfcb_d = nc.dram_tensor("wfcb", [128, 64], BF16, kind="ExternalInput")
    bfc_d = nc.dram_tensor("bfc", [1, 64], F32, kind="ExternalInput")
    id128_d = nc.dram_tensor("id128_in", [128, 128], F32, kind="ExternalInput")
    ones_d = nc.dram_tensor("ones_in", [1, 512], F32, kind="ExternalInput")
    zero8_d = nc.dram_tensor("zero8_in", [128, 8], BF16, kind="ExternalInput")
    y_d = nc.dram_tensor("y", [64, NTOK], F32, kind="ExternalOutput")

    # ---------------- SBUF ----------------
    sb = nc.alloc_sbuf_tensor
    x_stage = sb("x_stage", [128, 4 * T], F32)
    X0 = sb("X0", [64, NTOK], BF16)
    XL = {1: sb("XL1", [128, 2 * NTOK], BF16), 2: sb("XL2", [128, 2 * NTOK], BF16)}
    y_s = sb("y_s", [64, NTOK], F32)

    whh = {(l, d): sb(f"whh{l}{d}_s", [128, 512], BF16) for l in (0, 1) for d in DIRS}
    wih0 = {d: sb(f"wih0{d}_s", [64, 512], BF16) for d in DIRS}
    wih1a = {d: sb(f"wih1a{d}_s", [128, 512], BF16) for d in DIRS}
    wih1b = {d: sb(f"wih1b{d}_s", [128, 512], BF16) for d in DIRS}
    b4 = {(l, d): sb(f"b4_{l}{d}_s", [4, 128], BF16) for l in (0, 1) for d in DIRS}
    mask4 = sb("mask4", [4, 512], BF16)
    wfca = sb("wfca_s", [128, 64], BF16)
    wfcb = sb("wfcb_s", [128, 64], BF16)
    bfc = sb("bfc_s", [1, 64], F32)
    id128 = sb("id128", [128, 128], F32)
    ones_fc = sb("ones_fc", [1, 512], F32)
    zero8 = sb("zero8", [128, 8], BF16)

    # u[(d,par)]: cols 0:32 = sigma(gates) [i f g o]; cols 32:40 = C state
    u = {(d, p): sb(f"u_{d}{p}", [128, 40], F32) for d in DIRS for p in (0, 1)}
    pq = {d: sb(f"pq_{d}", [128, 16], F32) for d in DIRS}
    vhat = {d: sb(f"vhat_{d}", [128, 8], F32) for d in DIRS}

    gb = {(d, i): nc.alloc_psum_tensor(f"gb_{d}{i}", [128, 512], F32)
          for d in DIRS for i in (0, 1)}
    tbank = [nc.alloc_psum_tensor(f"tb{i}", [64, 512], F32) for i in range(2)]

    sem_in = nc.alloc_semaphore("sem_in")
    s_mm = nc.alloc_semaphore("s_mm")
    s_act = nc.alloc_semaphore("s_act")
    s_dve = nc.alloc_semaphore("s_dve")
    s_out = nc.alloc_semaphore("s_out")
    cnt = {"mm": 0, "act": 0, "dve": 0}
    sems = {"mm": s_mm, "act": s_act, "dve": s_dve}

    def W(eng, which, val):
        eng.wait_ge(sems[which], val)

    def inc(ins, which):
        ins.then_inc(sems[which], 1)
        cnt[which] += 1
        return cnt[which]

    # ---------------- input DMAs ----------------
    n_dma = 0

    def dma(dst, src):
        nonlocal n_dma
        nc.sync.dma_start(out=dst, in_=src).then_inc(sem_in, 16)
        n_dma += 1

    dma(x_stage[:, :], x_d[:].rearrange("b t d -> (b t d)").rearrange("(p f) -> p f", p=128))
    for l in (0, 1):
        for d in DIRS:
            dma(whh[(l, d)][:, :], whh_d[(l, d)][:, :])
            dma(b4[(l, d)][:, :], b4_d[(l, d)][:, :])
    for d in DIRS:
        dma(wih0[d][:, :], wih0_d[d][:, :])
        dma(wih1a[d][:, :], wih1a_d[d][:, :])
        dma(wih1b[d][:, :], wih1b_d[d][:, :])
    dma(mask4[:, :], mask4_d[:, :])
    dma(wfca[:, :], wfca_d[:, :])
    dma(wfcb[:, :], wfcb_d[:, :])
    dma(bfc[:, :], bfc_d[:, :])
    dma(id128[:, :], id128_d[:, :])
    dma(ones_fc[:, :], ones_d[:, :])
    dma(zero8[:, :], zero8_d[:, :])

    nc.tensor.wait_ge(sem_in, 16 * n_dma)

    # ---------------- x transpose into X0 ----------------
    TL = T // 16
    copy_done = {}
    for tlo in range(TL):
        bank = tbank[tlo % 2]
        if tlo >= 2:
            eng, c0 = copy_done[tlo - 2]
            W(nc.tensor, eng, c0)
        ins = nc.tensor.transpose(bank[0:64, 0:128],
                                  x_stage[:, tlo * 64:(tlo + 1) * 64], id128[:, :])
        inc(ins, "mm")
        trc = cnt["mm"]
        src = ap_of(bank, 0, [[pstride(bank), 64], [1, 16], [16, 8]])
        dst = ap_of(X0, tlo * 8, [[pstride(X0), 64], [TL * 8, 16], [1, 8]])
        if tlo % 4 < 2:
            W(nc.scalar, "mm", trc)
            ins = nc.scalar.activation(dst, src, ActFn.Copy)
            copy_done[tlo] = ("act", inc(ins, "act"))
        else:
            W(nc.vector, "mm", trc)
            ins = nc.vector.tensor_copy(dst, src)
            copy_done[tlo] = ("dve", inc(ins, "dve"))

    # ---------------- BiLSTM layers ----------------
    def pregate(l, d, c):
        """Emit bias + x-part matmuls for chunk c (steps 16c..16c+15) of dir d."""
        if c >= NCH:
            return
        bank = gb[(d, c % 2)]
        nc.tensor.matmul(bank[:, 0:512], b4[(l, d)][:, :], mask4[:, :],
                         start=True, stop=False, skip_group_check=True)
        if l == 0:
            parts = [(wih0[d], X0, 64, 0)]
        else:
            parts = [(wih1a[d], XL[1], 128, 0), (wih1b[d], XL[1], 128, NTOK)]
        for (lhsT, Xsrc, K, xoff) in parts:
            if d == "f":
                rhs = ap_of(Xsrc, xoff + c * 128, [[pstride(Xsrc), K], [1, 128]])
            else:
                # step j of chunk c handles time T-1-16c-j  -> negative stride
                rhs = ap_of(Xsrc, xoff + (T - 1 - 16 * c) * 8,
                            [[pstride(Xsrc), K], [-8, 16], [1, 8]])
            for g in range(4):
                dst = ap_of(bank, 8 * g, [[pstride(bank), 128], [32, 16], [1, 8]])
                nc.tensor.matmul(dst, lhsT[0:K, g * 128:(g + 1) * 128], rhs,
                                 start=False, stop=False, skip_group_check=True)

    def layer(l, Xout):
        # barrier: inputs (X0 or XL1) fully written
        W(nc.tensor, "act", cnt["act"])
        W(nc.tensor, "dve", cnt["dve"])
        # C state init: C = c/2 + 0.5 = 0.5 in u[(d,0)][:,32:40]
        for d in DIRS:
            nc.vector.memset(u[(d, 0)][:, 32:40], 0.5)
        for d in DIRS:
            pregate(l, d, 0)

        mm_done = {}
        sg_done = {}
        c_done = {}
        sc_done = {}
        h_done = {}

        for s in range(T):
            par = s % 2
            base = 32 * (s % 16)
            # ---- PE: rec matmuls fwd then bwd ----
            for d in DIRS:
                bank = gb[(d, (s // 16) % 2)]
                if s == 0:
                    rhs = zero8[:, :]
                else:
                    W(nc.tensor, "dve", h_done[(d, s - 1)])
                    if d == "f":
                        rhs = Xout[:, (s - 1) * 8: s * 8]
                    else:
                        rhs = ap_of(Xout, NTOK + (T - s) * 8,
                                    [[pstride(Xout), 128], [1, 8]])
                last = None
                for g in range(4):
                    last = nc.tensor.matmul(
                        bank[:, base + 8 * g: base + 8 * g + 8],
                        whh[(l, d)][:, g * 128:(g + 1) * 128], rhs,
                        start=False, stop=True, skip_group_check=True)
                mm_done[(d, s)] = inc(last, "mm")
            # ---- PE: pregate burst for chunk c+1 (safe: the target bank's
            # last sigma read was step 16c-1, ordered before via h-dep) ----
            if s % 16 == 0:
                pregate(l, "f", s // 16 + 1)
            elif s % 16 == 1:
                pregate(l, "b", s // 16 + 1)
            # ---- ACT: sigma over gates (both dirs) ----
            for d in DIRS:
                bank = gb[(d, (s // 16) % 2)]
                W(nc.scalar, "mm", mm_done[(d, s)])
                ins = nc.scalar.activation(u[(d, par)][:, 0:32],
                                           bank[:, base:base + 32], ActFn.Sigmoid)
                sg_done[(d, s)] = inc(ins, "act")
            # ---- DVE: PQ, C, h ----
            for d in DIRS:
                W(nc.vector, "act", sg_done[(d, s)])
                in0 = ap_of(u[(d, par)], 16, [[pstride(u[(d, par)]), 128], [16, 2], [1, 8]])
                in1 = ap_of(u[(d, par)], 0, [[pstride(u[(d, par)]), 128], [8, 2], [1, 8]])
                out = ap_of(pq[d], 0, [[pstride(pq[d]), 128], [8, 2], [1, 8]])
                nc.vector.scalar_tensor_tensor(out=out, in0=in0, scalar=0.5,
                                               in1=in1, op0=AluOp.subtract,
                                               op1=AluOp.mult)
            for d in DIRS:
                ins = nc.vector.scalar_tensor_tensor(
                    out=u[(d, 1 - par)][:, 32:40], in0=pq[d][:, 0:8], scalar=0.5,
                    in1=pq[d][:, 8:16], op0=AluOp.add, op1=AluOp.add)
                c_done[(d, s)] = inc(ins, "dve")
            # ---- ACT: v^ = sigma(4C-2) ----
            for d in DIRS:
                W(nc.scalar, "dve", c_done[(d, s)])
                ins = nc.scalar.activation(vhat[d][:, :], u[(d, 1 - par)][:, 32:40],
                                           ActFn.Sigmoid, scale=4.0, bias=-2.0)
                sc_done[(d, s)] = inc(ins, "act")
            # ---- DVE: h~ = (v^-0.5)*o~ ----
            for d in DIRS:
                W(nc.vector, "act", sc_done[(d, s)])
                if d == "f":
                    dst = Xout[:, s * 8:(s + 1) * 8]
                else:
                    dst = ap_of(Xout, NTOK + (T - 1 - s) * 8,
                                [[pstride(Xout), 128], [1, 8]])
                ins = nc.vector.scalar_tensor_tensor(
                    out=dst, in0=vhat[d][:, :], scalar=0.5,
                    in1=u[(d, par)][:, 24:32], op0=AluOp.subtract, op1=AluOp.mult)
                h_done[(d, s)] = inc(ins, "dve")

    layer(0, XL[1])
    layer(1, XL[2])

    # ---------------- FC ----------------
    W(nc.tensor, "act", cnt["act"])
    W(nc.tensor, "dve", cnt["dve"])
    fc_copy = {}
    fc_starts = list(range(0, NTOK, 512))
    for i, st in enumerate(fc_starts):
        w = min(512, NTOK - st)
        bank = tbank[i % 2]
        if i >= 2:
            eng, c0 = fc_copy[i - 2]
            W(nc.tensor, eng, c0)
        nc.tensor.matmul(bank[0:64, 0:w], bfc[:, :], ones_fc[:, 0:w],
                         start=True, stop=False, skip_group_check=True)
        nc.tensor.matmul(bank[0:64, 0:w], wfca[:, :], XL[2][:, st:st + w],
                         start=False, stop=False, skip_group_check=True)
        ins = nc.tensor.matmul(bank[0:64, 0:w], wfcb[:, :],
                               ap_of(XL[2], NTOK + st, [[pstride(XL[2]), 128], [1, w]]),
                               start=False, stop=True, skip_group_check=True)
        mmc = inc(ins, "mm")
        if i % 2 == 0:
            W(nc.scalar, "mm", mmc)
            ins = nc.scalar.activation(y_s[:, st:st + w], bank[0:64, 0:w], ActFn.Copy)
            fc_copy[i] = ("act", inc(ins, "act"))
        else:
            W(nc.vector, "mm", mmc)
            ins = nc.vector.tensor_copy(y_s[:, st:st + w], bank[0:64, 0:w])
            fc_copy[i] = ("dve", inc(ins, "dve"))

    # ---------------- output DMA ----------------
    nc.sync.wait_ge(s_act, cnt["act"])
    nc.sync.wait_ge(s_dve, cnt["dve"])
    nc.sync.dma_start(out=y_d[:, :], in_=y_s[:, :]).then_inc(s_out, 16)
    nc.sync.wait_ge(s_out, 16)
    return nc


# ====================== host-side prep & entry point ======================

def _to_bf(a):
    return np.asarray(a, dtype=np.float32).astype(BF)


def prep_weights(inputs):
    """Pre-scaled lhsT tensors per the v2 formulation."""
    out = {}
    for l in (0, 1):
        xin_scale = 1.0 if l == 0 else 2.0
        for dname, suf in (("f", ""), ("b", "r")):
            wih = np.asarray(inputs[f"w_ih_l{l}{suf}"], np.float32)   # [512, Din]
            whh = np.asarray(inputs[f"w_hh_l{l}{suf}"], np.float32)   # [512, 128]
            bsum = (np.asarray(inputs[f"b_ih_l{l}{suf}"], np.float32)
                    + np.asarray(inputs[f"b_hh_l{l}{suf}"], np.float32))
            blk_ih, blk_hh, b4 = [], [], np.zeros((4, 128), np.float32)
            for gi, G in enumerate(GO):
                rows = slice(PT[G] * 128, (PT[G] + 1) * 128)
                sG = 2.0 if G == "g" else 1.0
                blk_ih.append((sG * xin_scale * wih[rows]).T)   # [Din,128]
                blk_hh.append((sG * 2.0 * whh[rows]).T)         # [128,128]
                b4[gi] = sG * bsum[rows]
            wih_cat = np.concatenate(blk_ih, axis=1)            # [Din, 512]
            out[f"whh{l}{dname}"] = _to_bf(np.concatenate(blk_hh, axis=1))
            out[f"b4_{l}{dname}"] = _to_bf(b4)
            if l == 0:
                out[f"wih0{dname}"] = _to_bf(wih_cat)           # [64, 512]
            else:
                out[f"wih1a{dname}"] = _to_bf(wih_cat[0:128])
                out[f"wih1b{dname}"] = _to_bf(wih_cat[128:256])
    wfc = 2.0 * np.asarray(inputs["w_fc"], np.float32).T        # [256, 64]
    out["wfca"] = _to_bf(wfc[0:128])
    out["wfcb"] = _to_bf(wfc[128:256])
    out["bfc"] = np.asarray(inputs["b_fc"], np.float32).reshape(1, 64)
    return out


def _mask4_np():
    m = np.zeros((4, 512), np.float32)
    for g in range(4):
        for r in range(16):
            m[g, r * 32 + g * 8: r * 32 + g * 8 + 8] = 1.0
    return m.astype(BF)


_NC_CACHE = {}


def _get_nc(T):
    if T not in _NC_CACHE:
        _NC_CACHE[T] = build_nc(T)
    return _NC_CACHE[T]


def run_cores(inputs, T=512, n_cores=8, trace=False):
    x = np.asarray(inputs["x"], np.float32)
    per = 8
    wp = prep_weights(inputs)
    common = dict(wp)
    common.update({
        "mask4_in": _mask4_np(), "id128_in": np.eye(128, dtype=np.float32),
        "ones_in": np.ones((1, 512), np.float32),
        "zero8_in": np.zeros((128, 8), np.float32).astype(BF),
    })
    in_maps = []
    for c in range(n_cores):
        m = {}
        for l in (0, 1):
            for d in DIRS:
                m[f"whh{l}{d}"] = common[f"whh{l}{d}"]
                m[f"b4_{l}{d}"] = common[f"b4_{l}{d}"]
        for d in DIRS:
            m[f"wih0{d}"] = common[f"wih0{d}"]
            m[f"wih1a{d}"] = common[f"wih1a{d}"]
            m[f"wih1b{d}"] = common[f"wih1b{d}"]
        for k in ("wfca", "wfcb", "bfc", "mask4_in", "id128_in", "ones_in", "zero8_in"):
            m[k] = common[k]
        m["x"] = np.ascontiguousarray(x[c * per:(c + 1) * per, :T])
        in_maps.append(m)

    nc = _get_nc(T)
    res = run_bass_kernel_spmd(nc, in_maps, core_ids=list(range(n_cores)), trace=trace)
    outs = []
    for c in range(n_cores):
        yc = res.results[c]["y"]
        outs.append(yc.reshape(64, T, 8).transpose(2, 1, 0))
    return np.concatenate(outs, axis=0), res


def kernel(**inputs):
    y, _ = run_cores(inputs, T=512, n_cores=8)
    return y.astype(np.float32)


# revision 3
# speedup vs baseline: 1212.2908x; 1.0839x over previous
"""Trainium2 Bass kernel for nn_BiLSTM: 2-layer BiLSTM (B=64,T=512,D=64,H=128) + FC.

Sharding: data-parallel over batch across 8 NeuronCores (8 samples/core).

v2 design (split-dir, in-phase lockstep, min-chain cell update):
  Per layer, fwd and bwd run as two independent recurrent chains advanced in
  lockstep; per step s each dir does:
    PE : 4 gate matmuls  gates += Whh_g @ h~(s-1)    [128,8] into PSUM bank
    ACT: sg = Sigmoid(bank[32 cols])          -> u[par][0:32]   (i,f,g,o x 8)
    DVE: PQ = (in0 - .5) * in1  where in0=[g~|C], in1=[i~|f~]   -> [p|q]
    DVE: C' = (p + .5) + q                    -> u[par^1][32:40]
    ACT: v^ = Sigmoid(4C' - 2)                -> vhat
    DVE: h~ = (v^ - .5) * o~                  -> X[l] column (bf16)
  State: C = c/2 + 0.5 kept adjacent to the sigma outputs so PQ is one
  strided-AP op.  h~ = h/2; consumers (Whh, Wih_l1, Wfc) pre-scaled by 2.
  g-gate rows pre-scaled by 2 so one Sigmoid covers tanh(g) via 2sig(2g)-1.

  Engine queue order per step: PE [mmf x4, mmb x4, pregate burst]
                               ACT [sgf, sgb, scf, scb]
                               DVE [PQf, PQb, Cf, Cb, hf, hb]
  Pregates (x-part + bias) accumulate into 2 PSUM banks/dir (16 steps each),
  staged 2 chunks ahead during PE idle; no explicit waits needed (transitively
  ordered through the h-dependency).
Host: reshape y -> [8,T,64] per core, concat cores -> [64,T,64].
"""
import sys
sys.path.insert(0, "/opt/trn_rl_repo")
import numpy as np
import ml_dtypes

import concourse.bass as bass
from concourse import mybir
from concourse.bass_utils import run_bass_kernel_spmd

F32 = mybir.dt.float32
BF16 = mybir.dt.bfloat16
BF = ml_dtypes.bfloat16
AluOp = mybir.AluOpType
ActFn = mybir.ActivationFunctionType

H = 128
PT = {"i": 0, "f": 1, "g": 2, "o": 3}   # PyTorch row-block order
GO = ["i", "f", "g", "o"]               # PSUM/u col-block order (8 cols each)
DIRS = ("f", "b")


def ap_of(t, off, dims):
    base = t[:] if not isinstance(t, bass.AP) else t
    return bass.AP(tensor=base.tensor, offset=base.offset + off, ap=list(dims))


def pstride(t):
    base = t[:] if not isinstance(t, bass.AP) else t
    return base.ap[0][0]


def build_nc(T=512):
    assert T % 16 == 0
    NTOK = T * 8
    NCH = T // 16                      # pregate chunks (16 steps each)
    nc = bass.Bass("TRN2", target_bir_lowering=False, debug=False)

    # register -2.0 const AP (sigma-cell bias), same pattern as Bass.__init__
    _c = nc.alloc_sbuf_tensor("const-f32-neg2", [128, 1], F32)
    nc.gpsimd.memset(_c.ap(), -2.0)
    nc.const_aps.aps[(F32, -2.0)] = _c.ap()
    nc.all_engine_barrier()

    # ---------------- DRAM I/O ----------------
    x_d = nc.dram_tensor("x", [8, T, 64], F32, kind="ExternalInput")
    whh_d = {(l, d): nc.dram_tensor(f"whh{l}{d}", [128, 512], BF16, kind="ExternalInput")
             for l in (0, 1) for d in DIRS}
    wih0_d = {d: nc.dram_tensor(f"wih0{d}", [64, 512], BF16, kind="ExternalInput") for d in DIRS}
    wih1a_d = {d: nc.dram_tensor(f"wih1a{d}", [128, 512], BF16, kind="ExternalInput") for d in DIRS}
    wih1b_d = {d: nc.dram_tensor(f"wih1b{d}", [128, 512], BF16, kind="ExternalInput") for d in DIRS}
    b4_d = {(l, d): nc.dram_tensor(f"b4_{l}{d}", [4, 128], BF16, kind="ExternalInput")
            for l in (0, 1) for d in DIRS}
    mask4_d = nc.dram_tensor("mask4_in", [4, 512], BF16, kind="ExternalInput")
    wfca_d = nc.dram_tensor("wfca", [128, 64], BF16, kind="ExternalInput")
    wfcb_d = nc.dram_tensor("wfcb", [128, 64], BF16, kind="ExternalInput")
    bfc_d = nc.dram_tensor("bfc", [1, 64], F32, kind="ExternalInput")
    id128_d = nc.dram_tensor("id128_in", [128, 128], F32, kind="ExternalInput")
    ones_d = nc.dram_tensor("ones_in", [1, 512], F32, kind="ExternalInput")
    zero8_d = nc.dram_tensor("zero8_in", [128, 8], BF16, kind="ExternalInput")
    y_d = nc.dram_tensor("y", [64, NTOK], F32, kind="ExternalOutput")

    # ---------------- SBUF ----------------
    sb = nc.alloc_sbuf_tensor
    x_stage = sb("x_stage", [128, 4 * T], F32)
    X0 = sb("X0", [64, NTOK], BF16)
    XL = {1: sb("XL1", [128, 2 * NTOK], BF16), 2: sb("XL2", [128, 2 * NTOK], BF16)}
    y_s = sb("y_s", [64, NTOK], F32)

    whh = {(l, d): sb(f"whh{l}{d}_s", [128, 512], BF16) for l in (0, 1) for d in DIRS}
    wih0 = {d: sb(f"wih0{d}_s", [64, 512], BF16) for d in DIRS}
    wih1a = {d: sb(f"wih1a{d}_s", [128, 512], BF16) for d in DIRS}
    wih1b = {d: sb(f"wih1b{d}_s", [128, 512], BF16) for d in DIRS}
    b4 = {(l, d): sb(f"b4_{l}{d}_s", [4, 128], BF16) for l in (0, 1) for d in DIRS}
    mask4 = sb("mask4", [4, 512], BF16)
    wfca = sb("wfca_s", [128, 64], BF16)
    wfcb = sb("wfcb_s", [128, 64], BF16)
    bfc = sb("bfc_s", [1, 64], F32)
    id128 = sb("id128", [128, 128], F32)
    ones_fc = sb("ones_fc", [1, 512], F32)
    zero8 = sb("zero8", [128, 8], BF16)

    # u[(d,par)]: cols 0:32 = sigma(gates) [i f g o]; cols 32:40 = C state
    u = {(d, p): sb(f"u_{d}{p}", [128, 40], F32) for d in DIRS for p in (0, 1)}
    pq = {d: sb(f"pq_{d}", [128, 16], F32) for d in DIRS}
    vhat = {d: sb(f"vhat_{d}", [128, 8], F32) for d in DIRS}
    dummy = sb("dummy_sp", [128, 1], F32)

    gb = {(d, i): nc.alloc_psum_tensor(f"gb_{d}{i}", [128, 512], F32)
          for d in DIRS for i in (0, 1)}
    tbank = [nc.alloc_psum_tensor(f"tb{i}", [64, 512], F32) for i in range(2)]

    sem_in = nc.alloc_semaphore("sem_in")
    s_mm = nc.alloc_semaphore("s_mm")
    s_act = nc.alloc_semaphore("s_act")
    s_dve = nc.alloc_semaphore("s_dve")
    s_out = nc.alloc_semaphore("s_out")
    cnt = {"mm": 0, "act": 0, "dve": 0}
    sems = {"mm": s_mm, "act": s_act, "dve": s_dve}

    def W(eng, which, val):
        eng.wait_ge(sems[which], val)

    def inc(ins, which):
        ins.then_inc(sems[which], 1)
        cnt[which] += 1
        return cnt[which]

    # ---------------- input DMAs ----------------
    n_dma = 0

    def dma(dst, src):
        nonlocal n_dma
        nc.sync.dma_start(out=dst, in_=src).then_inc(sem_in, 16)
        n_dma += 1

    dma(x_stage[:, :], x_d[:].rearrange("b t d -> (b t d)").rearrange("(p f) -> p f", p=128))
    for l in (0, 1):
        for d in DIRS:
            dma(whh[(l, d)][:, :], whh_d[(l, d)][:, :])
            dma(b4[(l, d)][:, :], b4_d[(l, d)][:, :])
    for d in DIRS:
        dma(wih0[d][:, :], wih0_d[d][:, :])
        dma(wih1a[d][:, :], wih1a_d[d][:, :])
        dma(wih1b[d][:, :], wih1b_d[d][:, :])
    dma(mask4[:, :], mask4_d[:, :])
    dma(wfca[:, :], wfca_d[:, :])
    dma(wfcb[:, :], wfcb_d[:, :])
    dma(bfc[:, :], bfc_d[:, :])
    dma(id128[:, :], id128_d[:, :])
    dma(ones_fc[:, :], ones_d[:, :])
    dma(zero8[:, :], zero8_d[:, :])

    nc.tensor.wait_ge(sem_in, 16 * n_dma)

    # ---------------- x transpose into X0 ----------------
    TL = T // 16
    copy_done = {}
    for tlo in range(TL):
        bank = tbank[tlo % 2]
        if tlo >= 2:
            eng, c0 = copy_done[tlo - 2]
            W(nc.tensor, eng, c0)
        ins = nc.tensor.transpose(bank[0:64, 0:128],
                                  x_stage[:, tlo * 64:(tlo + 1) * 64], id128[:, :])
        inc(ins, "mm")
        trc = cnt["mm"]
        src = ap_of(bank, 0, [[pstride(bank), 64], [1, 16], [16, 8]])
        dst = ap_of(X0, tlo * 8, [[pstride(X0), 64], [TL * 8, 16], [1, 8]])
        if tlo % 4 < 2:
            W(nc.scalar, "mm", trc)
            ins = nc.scalar.activation(dst, src, ActFn.Copy)
            copy_done[tlo] = ("act", inc(ins, "act"))
        else:
            W(nc.vector, "mm", trc)
            ins = nc.vector.tensor_copy(dst, src)
            copy_done[tlo] = ("dve", inc(ins, "dve"))

    # ---------------- BiLSTM layers ----------------
    def pregate(l, d, c):
        """Emit bias + x-part matmuls for chunk c (steps 16c..16c+15) of dir d."""
        if c >= NCH:
            return
        bank = gb[(d, c % 2)]
        nc.tensor.matmul(bank[:, 0:512], b4[(l, d)][:, :], mask4[:, :],
                         start=True, stop=False, skip_group_check=True)
        if l == 0:
            parts = [(wih0[d], X0, 64, 0)]
        else:
            parts = [(wih1a[d], XL[1], 128, 0), (wih1b[d], XL[1], 128, NTOK)]
        for (lhsT, Xsrc, K, xoff) in parts:
            if d == "f":
                rhs = ap_of(Xsrc, xoff + c * 128, [[pstride(Xsrc), K], [1, 128]])
            else:
                # step j of chunk c handles time T-1-16c-j  -> negative stride
                rhs = ap_of(Xsrc, xoff + (T - 1 - 16 * c) * 8,
                            [[pstride(Xsrc), K], [-8, 16], [1, 8]])
            for g in range(4):
                dst = ap_of(bank, 8 * g, [[pstride(bank), 128], [32, 16], [1, 8]])
                nc.tensor.matmul(dst, lhsT[0:K, g * 128:(g + 1) * 128], rhs,
                                 start=False, stop=False, skip_group_check=True)

    def layer(l, Xout):
        # barrier: inputs (X0 or XL1) fully written
        W(nc.tensor, "act", cnt["act"])
        W(nc.tensor, "dve", cnt["dve"])
        # C state init: C = c/2 + 0.5 = 0.5 in u[(d,0)][:,32:40]
        for d in DIRS:
            nc.vector.memset(u[(d, 0)][:, 32:40], 0.5)
        for d in DIRS:
            pregate(l, d, 0)

        mm_done = {}
        sg_done = {}
        c_done = {}
        sc_done = {}
        h_done = {}

        for s in range(T):
            par = s % 2
            base = 32 * (s % 16)
            # ---- PE: rec matmuls fwd then bwd ----
            for d in DIRS:
                bank = gb[(d, (s // 16) % 2)]
                if s == 0:
                    rhs = zero8[:, :]
                else:
                    W(nc.tensor, "dve", h_done[(d, s - 1)])
                    if d == "f":
                        rhs = Xout[:, (s - 1) * 8: s * 8]
                    else:
                        rhs = ap_of(Xout, NTOK + (T - s) * 8,
                                    [[pstride(Xout), 128], [1, 8]])
                last = None
                for g in range(4):
                    last = nc.tensor.matmul(
                        bank[:, base + 8 * g: base + 8 * g + 8],
                        whh[(l, d)][:, g * 128:(g + 1) * 128], rhs,
                        start=False, stop=True, skip_group_check=True)
                mm_done[(d, s)] = inc(last, "mm")
            # ---- PE: pregate burst for chunk c+1 (safe: the target bank's
            # last sigma read was step 16c-1, ordered before via h-dep) ----
            if s % 16 == 0:
                pregate(l, "f", s // 16 + 1)
            elif s % 16 == 1:
                pregate(l, "b", s // 16 + 1)
            # ---- ACT: sigma over gates (both dirs) ----
            for d in DIRS:
                bank = gb[(d, (s // 16) % 2)]
                W(nc.scalar, "mm", mm_done[(d, s)])
                ins = nc.scalar.activation(u[(d, par)][:, 0:32],
                                           bank[:, base:base + 32], ActFn.Sigmoid)
                sg_done[(d, s)] = inc(ins, "act")
            # ---- DVE: [PQf, spacer, Cf, PQb, spacer, Cb] ----
            # The spacer (a) satisfies the DVE gap-0 RAW hazard between PQ and
            # C and (b) keeps Cf ahead of PQb's semaphore wait in the queue so
            # the fwd chain does not detour through the bwd gates.
            for d in DIRS:
                W(nc.vector, "act", sg_done[(d, s)])
                in0 = ap_of(u[(d, par)], 16, [[pstride(u[(d, par)]), 128], [16, 2], [1, 8]])
                in1 = ap_of(u[(d, par)], 0, [[pstride(u[(d, par)]), 128], [8, 2], [1, 8]])
                out = ap_of(pq[d], 0, [[pstride(pq[d]), 128], [8, 2], [1, 8]])
                nc.vector.scalar_tensor_tensor(out=out, in0=in0, scalar=0.5,
                                               in1=in1, op0=AluOp.subtract,
                                               op1=AluOp.mult)
                nc.vector.memset(dummy[:, :], 0.0)
                ins = nc.vector.scalar_tensor_tensor(
                    out=u[(d, 1 - par)][:, 32:40], in0=pq[d][:, 0:8], scalar=0.5,
                    in1=pq[d][:, 8:16], op0=AluOp.add, op1=AluOp.add)
                c_done[(d, s)] = inc(ins, "dve")
            # ---- ACT: v^ = sigma(4C-2) ----
            for d in DIRS:
                W(nc.scalar, "dve", c_done[(d, s)])
                ins = nc.scalar.activation(vhat[d][:, :], u[(d, 1 - par)][:, 32:40],
                                           ActFn.Sigmoid, scale=4.0, bias=-2.0)
                sc_done[(d, s)] = inc(ins, "act")
            # ---- DVE: h~ = (v^-0.5)*o~ ----
            for d in DIRS:
                W(nc.vector, "act", sc_done[(d, s)])
                if d == "f":
                    dst = Xout[:, s * 8:(s + 1) * 8]
                else:
                    dst = ap_of(Xout, NTOK + (T - 1 - s) * 8,
                                [[pstride(Xout), 128], [1, 8]])
                ins = nc.vector.scalar_tensor_tensor(
                    out=dst, in0=vhat[d][:, :], scalar=0.5,
                    in1=u[(d, par)][:, 24:32], op0=AluOp.subtract, op1=AluOp.mult)
                h_done[(d, s)] = inc(ins, "dve")

    layer(0, XL[1])
    layer(1, XL[2])

    # ---------------- FC ----------------
    W(nc.tensor, "act", cnt["act"])
    W(nc.tensor, "dve", cnt["dve"])
    fc_copy = {}
    fc_starts = list(range(0, NTOK, 512))
    for i, st in enumerate(fc_starts):
        w = min(512, NTOK - st)
        bank = tbank[i % 2]
        if i >= 2:
            eng, c0 = fc_copy[i - 2]
            W(nc.tensor, eng, c0)
        nc.tensor.matmul(bank[0:64, 0:w], bfc[:, :], ones_fc[:, 0:w],
                         start=True, stop=False, skip_group_check=True)
        nc.tensor.matmul(bank[0:64, 0:w], wfca[:, :], XL[2][:, st:st + w],
                         start=False, stop=False, skip_group_check=True)
        ins = nc.tensor.matmul(bank[0:64, 0:w], wfcb[:, :],
                               ap_of(XL[2], NTOK + st, [[pstride(XL[2]), 128], [1, w]]),
                               start=False, stop=True, skip_group_check=True)
        mmc = inc(ins, "mm")
        if i % 2 == 0:
            W(nc.scalar, "mm", mmc)
            ins = nc.scalar.activation(y_s[:, st:st + w], bank[0:64, 0:w], ActFn.Copy)
            fc_copy[i] = ("act", inc(ins, "act"))
        else:
            W(nc.vector, "mm", mmc)
            ins = nc.vector.tensor_copy(y_s[:, st:st + w], bank[0:64, 0:w])
            fc_copy[i] = ("dve", inc(ins, "dve"))

    # ---------------- output DMA ----------------
    nc.sync.wait_ge(s_act, cnt["act"])
    nc.sync.wait_ge(s_dve, cnt["dve"])
    nc.sync.dma_start(out=y_d[:, :], in_=y_s[:, :]).then_inc(s_out, 16)
    nc.sync.wait_ge(s_out, 16)
    return nc


# ====================== host-side prep & entry point ======================

def _to_bf(a):
    return np.asarray(a, dtype=np.float32).astype(BF)


def prep_weights(inputs):
    """Pre-scaled lhsT tensors per the v2 formulation."""
    out = {}
    for l in (0, 1):
        xin_scale = 1.0 if l == 0 else 2.0
        for dname, suf in (("f", ""), ("b", "r")):
            wih = np.asarray(inputs[f"w_ih_l{l}{suf}"], np.float32)   # [512, Din]
            whh = np.asarray(inputs[f"w_hh_l{l}{suf}"], np.float32)   # [512, 128]
            bsum = (np.asarray(inputs[f"b_ih_l{l}{suf}"], np.float32)
                    + np.asarray(inputs[f"b_hh_l{l}{suf}"], np.float32))
            blk_ih, blk_hh, b4 = [], [], np.zeros((4, 128), np.float32)
            for gi, G in enumerate(GO):
                rows = slice(PT[G] * 128, (PT[G] + 1) * 128)
                sG = 2.0 if G == "g" else 1.0
                blk_ih.append((sG * xin_scale * wih[rows]).T)   # [Din,128]
                blk_hh.append((sG * 2.0 * whh[rows]).T)         # [128,128]
                b4[gi] = sG * bsum[rows]
            wih_cat = np.concatenate(blk_ih, axis=1)            # [Din, 512]
            out[f"whh{l}{dname}"] = _to_bf(np.concatenate(blk_hh, axis=1))
            out[f"b4_{l}{dname}"] = _to_bf(b4)
            if l == 0:
                out[f"wih0{dname}"] = _to_bf(wih_cat)           # [64, 512]
            else:
                out[f"wih1a{dname}"] = _to_bf(wih_cat[0:128])
                out[f"wih1b{dname}"] = _to_bf(wih_cat[128:256])
    wfc = 2.0 * np.asarray(inputs["w_fc"], np.float32).T        # [256, 64]
    out["wfca"] = _to_bf(wfc[0:128])
    out["wfcb"] = _to_bf(wfc[128:256])
    out["bfc"] = np.asarray(inputs["b_fc"], np.float32).reshape(1, 64)
    return out


def _mask4_np():
    m = np.zeros((4, 512), np.float32)
    for g in range(4):
        for r in range(16):
            m[g, r * 32 + g * 8: r * 32 + g * 8 + 8] = 1.0
    return m.astype(BF)


_NC_CACHE = {}


def _get_nc(T):
    if T not in _NC_CACHE:
        _NC_CACHE[T] = build_nc(T)
    return _NC_CACHE[T]


def run_cores(inputs, T=512, n_cores=8, trace=False):
    x = np.asarray(inputs["x"], np.float32)
    per = 8
    wp = prep_weights(inputs)
    common = dict(wp)
    common.update({
        "mask4_in": _mask4_np(), "id128_in": np.eye(128, dtype=np.float32),
        "ones_in": np.ones((1, 512), np.float32),
        "zero8_in": np.zeros((128, 8), np.float32).astype(BF),
    })
    in_maps = []
    for c in range(n_cores):
        m = {}
        for l in (0, 1):
            for d in DIRS:
                m[f"whh{l}{d}"] = common[f"whh{l}{d}"]
                m[f"b4_{l}{d}"] = common[f"b4_{l}{d}"]
        for d in DIRS:
            m[f"wih0{d}"] = common[f"wih0{d}"]
            m[f"wih1a{d}"] = common[f"wih1a{d}"]
            m[f"wih1b{d}"] = common[f"wih1b{d}"]
        for k in ("wfca", "wfcb", "bfc", "mask4_in", "id128_in", "ones_in", "zero8_in"):
            m[k] = common[k]
        m["x"] = np.ascontiguousarray(x[c * per:(c + 1) * per, :T])
        in_maps.append(m)

    nc = _get_nc(T)
    res = run_bass_kernel_spmd(nc, in_maps, core_ids=list(range(n_cores)), trace=trace)
    outs = []
    for c in range(n_cores):
        yc = res.results[c]["y"]
        outs.append(yc.reshape(64, T, 8).transpose(2, 1, 0))
    return np.concatenate(outs, axis=0), res


def kernel(**inputs):
    y, _ = run_cores(inputs, T=512, n_cores=8)
    return y.astype(np.float32)


# revision 6
# speedup vs baseline: 1226.6020x; 1.0118x over previous
"""Trainium2 Bass kernel for nn_BiLSTM: 2-layer BiLSTM (B=64,T=512,D=64,H=128) + FC.

Sharding: data-parallel over batch across 8 NeuronCores (8 samples/core).

v2 design (split-dir, in-phase lockstep, min-chain cell update):
  Per layer, fwd and bwd run as two independent recurrent chains advanced in
  lockstep; per step s each dir does:
    PE : 4 gate matmuls  gates += Whh_g @ h~(s-1)    [128,8] into PSUM bank
    ACT: sg = Sigmoid(bank[32 cols])          -> u[par][0:32]   (i,f,g,o x 8)
    DVE: PQ = (in0 - .5) * in1  where in0=[g~|C], in1=[i~|f~]   -> [p|q]
    DVE: C' = (p + .5) + q                    -> u[par^1][32:40]
    ACT: v^ = Sigmoid(4C' - 2)                -> vhat
    DVE: h~ = (v^ - .5) * o~                  -> X[l] column (bf16)
  State: C = c/2 + 0.5 kept adjacent to the sigma outputs so PQ is one
  strided-AP op.  h~ = h/2; consumers (Whh, Wih_l1, Wfc) pre-scaled by 2.
  g-gate rows pre-scaled by 2 so one Sigmoid covers tanh(g) via 2sig(2g)-1.

  Engine queue order per step: PE [mmf x4, mmb x4, pregate burst]
                               ACT [sgf, sgb, scf, scb]
                               DVE [PQf, PQb, Cf, Cb, hf, hb]
  Pregates (x-part + bias) accumulate into 2 PSUM banks/dir (16 steps each),
  staged 2 chunks ahead during PE idle; no explicit waits needed (transitively
  ordered through the h-dependency).
Host: reshape y -> [8,T,64] per core, concat cores -> [64,T,64].
"""
import sys
sys.path.insert(0, "/opt/trn_rl_repo")
import numpy as np
import ml_dtypes

import concourse.bass as bass
from concourse import mybir
from concourse.bass_utils import run_bass_kernel_spmd

F32 = mybir.dt.float32
BF16 = mybir.dt.bfloat16
BF = ml_dtypes.bfloat16
AluOp = mybir.AluOpType
ActFn = mybir.ActivationFunctionType

H = 128
PT = {"i": 0, "f": 1, "g": 2, "o": 3}   # PyTorch row-block order
GO = ["i", "f", "g", "o"]               # PSUM/u col-block order (8 cols each)
DIRS = ("f", "b")


def ap_of(t, off, dims):
    base = t[:] if not isinstance(t, bass.AP) else t
    return bass.AP(tensor=base.tensor, offset=base.offset + off, ap=list(dims))


def pstride(t):
    base = t[:] if not isinstance(t, bass.AP) else t
    return base.ap[0][0]


def build_nc(T=512):
    assert T % 16 == 0
    NTOK = T * 8
    NCH = T // 16                      # pregate chunks (16 steps each)
    nc = bass.Bass("TRN2", target_bir_lowering=False, debug=False)

    # register -2.0 const AP (sigma-cell bias), same pattern as Bass.__init__
    _c = nc.alloc_sbuf_tensor("const-f32-neg2", [128, 1], F32)
    nc.gpsimd.memset(_c.ap(), -2.0)
    nc.const_aps.aps[(F32, -2.0)] = _c.ap()
    nc.all_engine_barrier()

    # ---------------- DRAM I/O (batched packs to cut DMA descriptors) ----
    x_d = nc.dram_tensor("x", [8, T, 64], F32, kind="ExternalInput")
    id128_d = nc.dram_tensor("id128_in", [128, 128], F32, kind="ExternalInput")
    # wpack cols: whh0f whh0b whh1f whh1b wih1af wih1ab wih1bf wih1bb (x512)
    wpack_d = nc.dram_tensor("wpack", [128, 4096], BF16, kind="ExternalInput")
    wih0p_d = nc.dram_tensor("wih0p", [64, 1024], BF16, kind="ExternalInput")
    # bpack cols: b4(0f) b4(0b) b4(1f) b4(1b) (x128), mask4 (512)
    bpack_d = nc.dram_tensor("bpack", [4, 1024], BF16, kind="ExternalInput")
    # fpack cols: wfca(64) wfcb(64) zero8(8)
    fpack_d = nc.dram_tensor("fpack", [128, 136], BF16, kind="ExternalInput")
    # onesp cols: ones(512) bfc(64)
    onesp_d = nc.dram_tensor("onesp", [1, 576], F32, kind="ExternalInput")
    y_d = nc.dram_tensor("y", [64, NTOK], F32, kind="ExternalOutput")

    # ---------------- SBUF ----------------
    sb = nc.alloc_sbuf_tensor
    x_stage = sb("x_stage", [128, 4 * T], F32)
    X0 = sb("X0", [64, NTOK], BF16)
    XL = {1: sb("XL1", [128, 2 * NTOK], BF16), 2: sb("XL2", [128, 2 * NTOK], BF16)}
    y_s = sb("y_s", [64, NTOK], F32)

    wpack = sb("wpack_s", [128, 4096], BF16)
    wih0p = sb("wih0p_s", [64, 1024], BF16)
    bpack = sb("bpack_s", [4, 1024], BF16)
    fpack = sb("fpack_s", [128, 136], BF16)
    onesp = sb("onesp_s", [1, 576], F32)
    id128 = sb("id128", [128, 128], F32)

    WOFF = {(0, "f"): 0, (0, "b"): 512, (1, "f"): 1024, (1, "b"): 1536}
    W1A = {"f": 2048, "b": 2560}
    W1B = {"f": 3072, "b": 3584}
    B4OFF = {(0, "f"): 0, (0, "b"): 128, (1, "f"): 256, (1, "b"): 384}

    def whh_slice(l, d, g):
        return ap_of(wpack, WOFF[(l, d)] + g * 128, [[pstride(wpack), 128], [1, 128]])

    def zero8_ap():
        return ap_of(fpack, 128, [[pstride(fpack), 128], [1, 8]])

    # u[(d,par)]: cols 0:32 = sigma(gates) [i f g o]; cols 32:40 = C state
    u = {(d, p): sb(f"u_{d}{p}", [128, 40], F32) for d in DIRS for p in (0, 1)}
    pq = {d: sb(f"pq_{d}", [128, 16], F32) for d in DIRS}
    vhat = {d: sb(f"vhat_{d}", [128, 8], F32) for d in DIRS}
    dummy = sb("dummy_sp", [128, 1], F32)

    gb = {(d, i): nc.alloc_psum_tensor(f"gb_{d}{i}", [128, 512], F32)
          for d in DIRS for i in (0, 1)}
    tbank = [nc.alloc_psum_tensor(f"tb{i}", [64, 512], F32) for i in range(4)]

    sem_in = nc.alloc_semaphore("sem_in")
    sem_pre = nc.alloc_semaphore("sem_pre")
    s_mm = nc.alloc_semaphore("s_mm")
    s_act = nc.alloc_semaphore("s_act")
    s_dve = nc.alloc_semaphore("s_dve")
    s_out = nc.alloc_semaphore("s_out")
    cnt = {"mm": 0, "act": 0, "dve": 0}
    sems = {"mm": s_mm, "act": s_act, "dve": s_dve}

    def W(eng, which, val):
        eng.wait_ge(sems[which], val)

    # pre-warm the sigmoid activation table set while input DMAs stream
    # (the first ACT instruction otherwise pays the ~2.7us table load on the
    # critical path; Copy lives in every set so it won't trigger a reload)
    nc.scalar.activation(dummy[:, :], dummy[:, :], ActFn.Sigmoid)

    def inc(ins, which):
        ins.then_inc(sems[which], 1)
        cnt[which] += 1
        return cnt[which]

    # ---------------- input DMAs ----------------
    n_dma = 0

    def dma(dst, src):
        nonlocal n_dma
        nc.sync.dma_start(out=dst, in_=src).then_inc(sem_in, 16)
        n_dma += 1

    # x + id128 on their own semaphore: the transpose preamble can start as
    # soon as THESE two land (DMA completions are not FIFO across descriptors)
    nc.sync.dma_start(out=x_stage[:, :],
                      in_=x_d[:].rearrange("b t d -> (b t d)").rearrange(
                          "(p f) -> p f", p=128)).then_inc(sem_pre, 16)
    nc.sync.dma_start(out=id128[:, :], in_=id128_d[:, :]).then_inc(sem_pre, 16)
    dma(wpack[:, :], wpack_d[:, :])
    dma(wih0p[:, :], wih0p_d[:, :])
    dma(bpack[:, :], bpack_d[:, :])
    dma(fpack[:, :], fpack_d[:, :])
    dma(onesp[:, :], onesp_d[:, :])

    nc.tensor.wait_ge(sem_pre, 32)

    # ---------------- x transpose into X0 ----------------
    TL = T // 16
    copy_done = {}
    for tlo in range(TL):
        bank = tbank[tlo % 4]
        if tlo >= 4:
            eng, c0 = copy_done[tlo - 4]
            W(nc.tensor, eng, c0)
        ins = nc.tensor.transpose(bank[0:64, 0:128],
                                  x_stage[:, tlo * 64:(tlo + 1) * 64], id128[:, :])
        inc(ins, "mm")
        trc = cnt["mm"]
        src = ap_of(bank, 0, [[pstride(bank), 64], [1, 16], [16, 8]])
        dst = ap_of(X0, tlo * 8, [[pstride(X0), 64], [TL * 8, 16], [1, 8]])
        if tlo % 4 < 2:
            W(nc.scalar, "mm", trc)
            ins = nc.scalar.activation(dst, src, ActFn.Copy)
            copy_done[tlo] = ("act", inc(ins, "act"))
        else:
            W(nc.vector, "mm", trc)
            ins = nc.vector.tensor_copy(dst, src)
            copy_done[tlo] = ("dve", inc(ins, "dve"))

    # ---------------- BiLSTM layers ----------------
    def pregate(l, d, c):
        """Emit bias + x-part matmuls for chunk c (steps 16c..16c+15) of dir d."""
        if c >= NCH:
            return
        bank = gb[(d, c % 2)]
        b4ap = ap_of(bpack, B4OFF[(l, d)], [[pstride(bpack), 4], [1, 128]])
        mask4ap = ap_of(bpack, 512, [[pstride(bpack), 4], [1, 512]])
        nc.tensor.matmul(bank[:, 0:512], b4ap, mask4ap,
                         start=True, stop=False, skip_group_check=True)
        if l == 0:
            parts = [(wih0p, 0 if d == "f" else 512, X0, 64, 0)]
        else:
            parts = [(wpack, W1A[d], XL[1], 128, 0),
                     (wpack, W1B[d], XL[1], 128, NTOK)]
        for (wt, woff, Xsrc, K, xoff) in parts:
            if d == "f":
                rhs = ap_of(Xsrc, xoff + c * 128, [[pstride(Xsrc), K], [1, 128]])
            else:
                # step j of chunk c handles time T-1-16c-j  -> negative stride
                rhs = ap_of(Xsrc, xoff + (T - 1 - 16 * c) * 8,
                            [[pstride(Xsrc), K], [-8, 16], [1, 8]])
            for g in range(4):
                dst = ap_of(bank, 8 * g, [[pstride(bank), 128], [32, 16], [1, 8]])
                lhsT = ap_of(wt, woff + g * 128, [[pstride(wt), K], [1, 128]])
                nc.tensor.matmul(dst, lhsT, rhs,
                                 start=False, stop=False, skip_group_check=True)

    def layer(l, Xout):
        # barrier: inputs (X0 or XL1) fully written; weights DMA'd
        if l == 0:
            nc.tensor.wait_ge(sem_in, 16 * n_dma)
        W(nc.tensor, "act", cnt["act"])
        W(nc.tensor, "dve", cnt["dve"])
        # C state init: C = c/2 + 0.5 = 0.5 in u[(d,0)][:,32:40]
        for d in DIRS:
            nc.vector.memset(u[(d, 0)][:, 32:40], 0.5)
        for d in DIRS:
            pregate(l, d, 0)

        mm_done = {}
        sg_done = {}
        c_done = {}
        sc_done = {}
        h_done = {}

        for s in range(T):
            par = s % 2
            base = 32 * (s % 16)
            # ---- PE: rec matmuls fwd then bwd ----
            for d in DIRS:
                bank = gb[(d, (s // 16) % 2)]
                if s == 0:
                    rhs = zero8_ap()
                else:
                    W(nc.tensor, "dve", h_done[(d, s - 1)])
                    if d == "f":
                        rhs = Xout[:, (s - 1) * 8: s * 8]
                    else:
                        rhs = ap_of(Xout, NTOK + (T - s) * 8,
                                    [[pstride(Xout), 128], [1, 8]])
                last = None
                for g in range(4):
                    last = nc.tensor.matmul(
                        bank[:, base + 8 * g: base + 8 * g + 8],
                        whh_slice(l, d, g), rhs,
                        start=False, stop=True, skip_group_check=True)
                mm_done[(d, s)] = inc(last, "mm")
            # ---- PE: pregate burst for chunk c+1 (safe: the target bank's
            # last sigma read was step 16c-1, ordered before via h-dep) ----
            if s % 16 == 0:
                pregate(l, "f", s // 16 + 1)
            elif s % 16 == 1:
                pregate(l, "b", s // 16 + 1)
            # ---- ACT: sigma over gates (both dirs) ----
            for d in DIRS:
                bank = gb[(d, (s // 16) % 2)]
                W(nc.scalar, "mm", mm_done[(d, s)])
                ins = nc.scalar.activation(u[(d, par)][:, 0:32],
                                           bank[:, base:base + 32], ActFn.Sigmoid)
                sg_done[(d, s)] = inc(ins, "act")
            # ---- DVE: [PQf, spacer, Cf, PQb, spacer, Cb] ----
            # The spacer (a) satisfies the DVE gap-0 RAW hazard between PQ and
            # C and (b) keeps Cf ahead of PQb's semaphore wait in the queue so
            # the fwd chain does not detour through the bwd gates.
            for d in DIRS:
                W(nc.vector, "act", sg_done[(d, s)])
                in0 = ap_of(u[(d, par)], 16, [[pstride(u[(d, par)]), 128], [16, 2], [1, 8]])
                in1 = ap_of(u[(d, par)], 0, [[pstride(u[(d, par)]), 128], [8, 2], [1, 8]])
                out = ap_of(pq[d], 0, [[pstride(pq[d]), 128], [8, 2], [1, 8]])
                nc.vector.scalar_tensor_tensor(out=out, in0=in0, scalar=0.5,
                                               in1=in1, op0=AluOp.subtract,
                                               op1=AluOp.mult)
                nc.vector.memset(dummy[:, :], 0.0)
                ins = nc.vector.scalar_tensor_tensor(
                    out=u[(d, 1 - par)][:, 32:40], in0=pq[d][:, 0:8], scalar=0.5,
                    in1=pq[d][:, 8:16], op0=AluOp.add, op1=AluOp.add)
                c_done[(d, s)] = inc(ins, "dve")
            # ---- ACT: v^ = sigma(4C-2) ----
            for d in DIRS:
                W(nc.scalar, "dve", c_done[(d, s)])
                ins = nc.scalar.activation(vhat[d][:, :], u[(d, 1 - par)][:, 32:40],
                                           ActFn.Sigmoid, scale=4.0, bias=-2.0)
                sc_done[(d, s)] = inc(ins, "act")
            # ---- DVE: h~ = (v^-0.5)*o~ ----
            for d in DIRS:
                W(nc.vector, "act", sc_done[(d, s)])
                if d == "f":
                    dst = Xout[:, s * 8:(s + 1) * 8]
                else:
                    dst = ap_of(Xout, NTOK + (T - 1 - s) * 8,
                                [[pstride(Xout), 128], [1, 8]])
                ins = nc.vector.scalar_tensor_tensor(
                    out=dst, in0=vhat[d][:, :], scalar=0.5,
                    in1=u[(d, par)][:, 24:32], op0=AluOp.subtract, op1=AluOp.mult)
                h_done[(d, s)] = inc(ins, "dve")

    layer(0, XL[1])
    layer(1, XL[2])

    # ---------------- FC (+ per-pair output DMA overlap) ----------------
    W(nc.tensor, "act", cnt["act"])
    W(nc.tensor, "dve", cnt["dve"])
    bfc_ap = ap_of(onesp, 512, [[pstride(onesp), 1], [1, 64]])
    ones_ap = ap_of(onesp, 0, [[pstride(onesp), 1], [1, 512]])
    wfca_ap = ap_of(fpack, 0, [[pstride(fpack), 128], [1, 64]])
    wfcb_ap = ap_of(fpack, 64, [[pstride(fpack), 128], [1, 64]])
    fc_copy = {}
    n_out = 0
    fc_starts = list(range(0, NTOK, 512))
    for i, st in enumerate(fc_starts):
        w = min(512, NTOK - st)
        bank = tbank[i % 4]
        if i >= 4:
            eng, c0 = fc_copy[i - 4]
            W(nc.tensor, eng, c0)
        nc.tensor.matmul(bank[0:64, 0:w], bfc_ap, ones_ap,
                         start=True, stop=False, skip_group_check=True)
        nc.tensor.matmul(bank[0:64, 0:w], wfca_ap, XL[2][:, st:st + w],
                         start=False, stop=False, skip_group_check=True)
        ins = nc.tensor.matmul(bank[0:64, 0:w], wfcb_ap,
                               ap_of(XL[2], NTOK + st, [[pstride(XL[2]), 128], [1, w]]),
                               start=False, stop=True, skip_group_check=True)
        mmc = inc(ins, "mm")
        if i % 2 == 0:
            W(nc.scalar, "mm", mmc)
            ins = nc.scalar.activation(y_s[:, st:st + w], bank[0:64, 0:w], ActFn.Copy)
            fc_copy[i] = ("act", inc(ins, "act"))
        else:
            W(nc.vector, "mm", mmc)
            ins = nc.vector.tensor_copy(y_s[:, st:st + w], bank[0:64, 0:w])
            fc_copy[i] = ("dve", inc(ins, "dve"))
        if i % 2 == 1:
            # DMA out the finished 1024-col slice while later chunks compute
            nc.sync.wait_ge(s_act, fc_copy[i - 1][1])
            nc.sync.wait_ge(s_dve, fc_copy[i][1])
            nc.sync.dma_start(out=y_d[:, st - 512:st + w],
                              in_=y_s[:, st - 512:st + w]).then_inc(s_out, 16)
            n_out += 1
    if len(fc_starts) % 2 == 1:
        st = fc_starts[-1]
        w = min(512, NTOK - st)
        nc.sync.wait_ge(s_act, cnt["act"])
        nc.sync.wait_ge(s_dve, cnt["dve"])
        nc.sync.dma_start(out=y_d[:, st:st + w],
                          in_=y_s[:, st:st + w]).then_inc(s_out, 16)
        n_out += 1

    nc.sync.wait_ge(s_out, 16 * n_out)
    return nc


# ====================== host-side prep & entry point ======================

def _to_bf(a):
    return np.asarray(a, dtype=np.float32).astype(BF)


def prep_weights(inputs):
    """Pre-scaled lhsT tensors per the v2 formulation."""
    out = {}
    for l in (0, 1):
        xin_scale = 1.0 if l == 0 else 2.0
        for dname, suf in (("f", ""), ("b", "r")):
            wih = np.asarray(inputs[f"w_ih_l{l}{suf}"], np.float32)   # [512, Din]
            whh = np.asarray(inputs[f"w_hh_l{l}{suf}"], np.float32)   # [512, 128]
            bsum = (np.asarray(inputs[f"b_ih_l{l}{suf}"], np.float32)
                    + np.asarray(inputs[f"b_hh_l{l}{suf}"], np.float32))
            blk_ih, blk_hh, b4 = [], [], np.zeros((4, 128), np.float32)
            for gi, G in enumerate(GO):
                rows = slice(PT[G] * 128, (PT[G] + 1) * 128)
                sG = 2.0 if G == "g" else 1.0
                blk_ih.append((sG * xin_scale * wih[rows]).T)   # [Din,128]
                blk_hh.append((sG * 2.0 * whh[rows]).T)         # [128,128]
                b4[gi] = sG * bsum[rows]
            wih_cat = np.concatenate(blk_ih, axis=1)            # [Din, 512]
            out[f"whh{l}{dname}"] = _to_bf(np.concatenate(blk_hh, axis=1))
            out[f"b4_{l}{dname}"] = _to_bf(b4)
            if l == 0:
                out[f"wih0{dname}"] = _to_bf(wih_cat)           # [64, 512]
            else:
                out[f"wih1a{dname}"] = _to_bf(wih_cat[0:128])
                out[f"wih1b{dname}"] = _to_bf(wih_cat[128:256])
    wfc = 2.0 * np.asarray(inputs["w_fc"], np.float32).T        # [256, 64]
    out["wfca"] = _to_bf(wfc[0:128])
    out["wfcb"] = _to_bf(wfc[128:256])
    out["bfc"] = np.asarray(inputs["b_fc"], np.float32).reshape(1, 64)
    return out


def _mask4_np():
    m = np.zeros((4, 512), np.float32)
    for g in range(4):
        for r in range(16):
            m[g, r * 32 + g * 8: r * 32 + g * 8 + 8] = 1.0
    return m.astype(BF)


_NC_CACHE = {}


def _get_nc(T):
    if T not in _NC_CACHE:
        _NC_CACHE[T] = build_nc(T)
    return _NC_CACHE[T]


def run_cores(inputs, T=512, n_cores=8, trace=False):
    x = np.asarray(inputs["x"], np.float32)
    per = 8
    wp = prep_weights(inputs)
    wpack = np.concatenate([wp["whh0f"], wp["whh0b"], wp["whh1f"], wp["whh1b"],
                            wp["wih1af"], wp["wih1ab"], wp["wih1bf"], wp["wih1bb"]],
                           axis=1)                                    # [128, 4096]
    wih0p = np.concatenate([wp["wih0f"], wp["wih0b"]], axis=1)        # [64, 1024]
    bpack = np.concatenate([wp["b4_0f"], wp["b4_0b"], wp["b4_1f"], wp["b4_1b"],
                            _mask4_np()], axis=1)                     # [4, 1024]
    fpack = np.concatenate([wp["wfca"], wp["wfcb"],
                            np.zeros((128, 8), BF)], axis=1)          # [128, 136]
    onesp = np.concatenate([np.ones((1, 512), np.float32),
                            wp["bfc"]], axis=1)                       # [1, 576]
    common = {
        "wpack": wpack, "wih0p": wih0p, "bpack": bpack, "fpack": fpack,
        "onesp": onesp, "id128_in": np.eye(128, dtype=np.float32),
    }
    in_maps = []
    for c in range(n_cores):
        m = dict(common)
        m["x"] = np.ascontiguousarray(x[c * per:(c + 1) * per, :T])
        in_maps.append(m)

    nc = _get_nc(T)
    res = run_bass_kernel_spmd(nc, in_maps, core_ids=list(range(n_cores)), trace=trace)
    outs = []
    for c in range(n_cores):
        yc = res.results[c]["y"]
        outs.append(yc.reshape(64, T, 8).transpose(2, 1, 0))
    return np.concatenate(outs, axis=0), res


def kernel(**inputs):
    y, _ = run_cores(inputs, T=512, n_cores=8)
    return y.astype(np.float32)


# revision 7
# speedup vs baseline: 1229.5752x; 1.0024x over previous
"""Trainium2 Bass kernel for nn_BiLSTM: 2-layer BiLSTM (B=64,T=512,D=64,H=128) + FC.

Sharding: data-parallel over batch across 8 NeuronCores (8 samples/core).

v2 design (split-dir, in-phase lockstep, min-chain cell update):
  Per layer, fwd and bwd run as two independent recurrent chains advanced in
  lockstep; per step s each dir does:
    PE : 4 gate matmuls  gates += Whh_g @ h~(s-1)    [128,8] into PSUM bank
    ACT: sg = Sigmoid(bank[32 cols])          -> u[par][0:32]   (i,f,g,o x 8)
    DVE: PQ = (in0 - .5) * in1  where in0=[g~|C], in1=[i~|f~]   -> [p|q]
    DVE: C' = (p + .5) + q                    -> u[par^1][32:40]
    ACT: v^ = Sigmoid(4C' - 2)                -> vhat
    DVE: h~ = (v^ - .5) * o~                  -> X[l] column (bf16)
  State: C = c/2 + 0.5 kept adjacent to the sigma outputs so PQ is one
  strided-AP op.  h~ = h/2; consumers (Whh, Wih_l1, Wfc) pre-scaled by 2.
  g-gate rows pre-scaled by 2 so one Sigmoid covers tanh(g) via 2sig(2g)-1.

  Engine queue order per step: PE [mmf x4, mmb x4, pregate burst]
                               ACT [sgf, sgb, scf, scb]
                               DVE [PQf, PQb, Cf, Cb, hf, hb]
  Pregates (x-part + bias) accumulate into 2 PSUM banks/dir (16 steps each),
  staged 2 chunks ahead during PE idle; no explicit waits needed (transitively
  ordered through the h-dependency).
Host: reshape y -> [8,T,64] per core, concat cores -> [64,T,64].
"""
import sys
sys.path.insert(0, "/opt/trn_rl_repo")
import numpy as np
import ml_dtypes

import concourse.bass as bass
from concourse import mybir
from concourse.bass_utils import run_bass_kernel_spmd

F32 = mybir.dt.float32
BF16 = mybir.dt.bfloat16
BF = ml_dtypes.bfloat16
AluOp = mybir.AluOpType
ActFn = mybir.ActivationFunctionType

H = 128
PT = {"i": 0, "f": 1, "g": 2, "o": 3}   # PyTorch row-block order
GO = ["i", "f", "g", "o"]               # PSUM/u col-block order (8 cols each)
DIRS = ("f", "b")


def ap_of(t, off, dims):
    base = t[:] if not isinstance(t, bass.AP) else t
    return bass.AP(tensor=base.tensor, offset=base.offset + off, ap=list(dims))


def pstride(t):
    base = t[:] if not isinstance(t, bass.AP) else t
    return base.ap[0][0]


def build_nc(T=512):
    assert T % 16 == 0
    NTOK = T * 8
    NCH = T // 16                      # pregate chunks (16 steps each)
    nc = bass.Bass("TRN2", target_bir_lowering=False, debug=False)

    # register -2.0 const AP (sigma-cell bias), same pattern as Bass.__init__
    _c = nc.alloc_sbuf_tensor("const-f32-neg2", [128, 1], F32)
    nc.gpsimd.memset(_c.ap(), -2.0)
    nc.const_aps.aps[(F32, -2.0)] = _c.ap()
    nc.all_engine_barrier()

    # ---------------- DRAM I/O (batched packs to cut DMA descriptors) ----
    # x0: host-pretransposed input  [64 d, t*8+b]  bf16
    x0_d = nc.dram_tensor("x0", [64, NTOK], BF16, kind="ExternalInput")
    # wpack cols: whh0f whh0b whh1f whh1b wih1af wih1ab wih1bf wih1bb (x512)
    wpack_d = nc.dram_tensor("wpack", [128, 4096], BF16, kind="ExternalInput")
    wih0p_d = nc.dram_tensor("wih0p", [64, 1024], BF16, kind="ExternalInput")
    # bpack cols: b4(0f) b4(0b) b4(1f) b4(1b) (x128), mask4 (512)
    bpack_d = nc.dram_tensor("bpack", [4, 1024], BF16, kind="ExternalInput")
    # fpack cols: wfca(64) wfcb(64) zero8(8)
    fpack_d = nc.dram_tensor("fpack", [128, 136], BF16, kind="ExternalInput")
    # onesp cols: ones(512) bfc(64)
    onesp_d = nc.dram_tensor("onesp", [1, 576], F32, kind="ExternalInput")
    y_d = nc.dram_tensor("y", [64, NTOK], F32, kind="ExternalOutput")

    # ---------------- SBUF ----------------
    sb = nc.alloc_sbuf_tensor
    X0 = sb("X0", [64, NTOK], BF16)
    XL = {1: sb("XL1", [128, 2 * NTOK], BF16), 2: sb("XL2", [128, 2 * NTOK], BF16)}
    y_s = sb("y_s", [64, NTOK], F32)

    wpack = sb("wpack_s", [128, 4096], BF16)
    wih0p = sb("wih0p_s", [64, 1024], BF16)
    bpack = sb("bpack_s", [4, 1024], BF16)
    fpack = sb("fpack_s", [128, 136], BF16)
    onesp = sb("onesp_s", [1, 576], F32)

    WOFF = {(0, "f"): 0, (0, "b"): 512, (1, "f"): 1024, (1, "b"): 1536}
    W1A = {"f": 2048, "b": 2560}
    W1B = {"f": 3072, "b": 3584}
    B4OFF = {(0, "f"): 0, (0, "b"): 128, (1, "f"): 256, (1, "b"): 384}

    def whh_slice(l, d, g):
        return ap_of(wpack, WOFF[(l, d)] + g * 128, [[pstride(wpack), 128], [1, 128]])

    def zero8_ap():
        return ap_of(fpack, 128, [[pstride(fpack), 128], [1, 8]])

    # u[(d,par)]: cols 0:32 = sigma(gates) [i f g o]; cols 32:40 = C state
    u = {(d, p): sb(f"u_{d}{p}", [128, 40], F32) for d in DIRS for p in (0, 1)}
    pq = {d: sb(f"pq_{d}", [128, 16], F32) for d in DIRS}
    vhat = {d: sb(f"vhat_{d}", [128, 8], F32) for d in DIRS}
    dummy = sb("dummy_sp", [128, 1], F32)

    gb = {(d, i): nc.alloc_psum_tensor(f"gb_{d}{i}", [128, 512], F32)
          for d in DIRS for i in (0, 1)}
    tbank = [nc.alloc_psum_tensor(f"tb{i}", [64, 512], F32) for i in range(4)]

    sem_in = nc.alloc_semaphore("sem_in")
    s_mm = nc.alloc_semaphore("s_mm")
    s_act = nc.alloc_semaphore("s_act")
    s_dve = nc.alloc_semaphore("s_dve")
    s_out = nc.alloc_semaphore("s_out")
    cnt = {"mm": 0, "act": 0, "dve": 0}
    sems = {"mm": s_mm, "act": s_act, "dve": s_dve}

    def W(eng, which, val):
        eng.wait_ge(sems[which], val)

    # pre-warm the sigmoid activation table set while input DMAs stream
    # (the first ACT instruction otherwise pays the ~2.7us table load on the
    # critical path; Copy lives in every set so it won't trigger a reload)
    nc.scalar.activation(dummy[:, :], dummy[:, :], ActFn.Sigmoid)

    def inc(ins, which):
        ins.then_inc(sems[which], 1)
        cnt[which] += 1
        return cnt[which]

    # ---------------- input DMAs ----------------
    n_dma = 0

    def dma(dst, src):
        nonlocal n_dma
        nc.sync.dma_start(out=dst, in_=src).then_inc(sem_in, 16)
        n_dma += 1

    dma(X0[:, :], x0_d[:, :])
    dma(wpack[:, :], wpack_d[:, :])
    dma(wih0p[:, :], wih0p_d[:, :])
    dma(bpack[:, :], bpack_d[:, :])
    dma(fpack[:, :], fpack_d[:, :])
    dma(onesp[:, :], onesp_d[:, :])

    # ---------------- BiLSTM layers ----------------
    def pregate_bias(l, d, c):
        if c >= NCH:
            return
        bank = gb[(d, c % 2)]
        b4ap = ap_of(bpack, B4OFF[(l, d)], [[pstride(bpack), 4], [1, 128]])
        mask4ap = ap_of(bpack, 512, [[pstride(bpack), 4], [1, 512]])
        nc.tensor.matmul(bank[:, 0:512], b4ap, mask4ap,
                         start=True, stop=False, skip_group_check=True)

    def pregate_part(l, d, c, pi):
        """x-part matmuls (part pi) for chunk c (steps 16c..16c+15) of dir d."""
        if c >= NCH:
            return
        bank = gb[(d, c % 2)]
        if l == 0:
            parts = [(wih0p, 0 if d == "f" else 512, X0, 64, 0)]
        else:
            parts = [(wpack, W1A[d], XL[1], 128, 0),
                     (wpack, W1B[d], XL[1], 128, NTOK)]
        if pi >= len(parts):
            return
        (wt, woff, Xsrc, K, xoff) = parts[pi]
        if d == "f":
            rhs = ap_of(Xsrc, xoff + c * 128, [[pstride(Xsrc), K], [1, 128]])
        else:
            # step j of chunk c handles time T-1-16c-j  -> negative stride
            rhs = ap_of(Xsrc, xoff + (T - 1 - 16 * c) * 8,
                        [[pstride(Xsrc), K], [-8, 16], [1, 8]])
        for g in range(4):
            dst = ap_of(bank, 8 * g, [[pstride(bank), 128], [32, 16], [1, 8]])
            lhsT = ap_of(wt, woff + g * 128, [[pstride(wt), K], [1, 128]])
            nc.tensor.matmul(dst, lhsT, rhs,
                             start=False, stop=False, skip_group_check=True)

    def pregate(l, d, c):
        pregate_bias(l, d, c)
        pregate_part(l, d, c, 0)
        pregate_part(l, d, c, 1)

    def layer(l, Xout):
        # barrier: inputs (X0 or XL1) fully written; weights DMA'd
        if l == 0:
            nc.tensor.wait_ge(sem_in, 16 * n_dma)
        W(nc.tensor, "act", cnt["act"])
        W(nc.tensor, "dve", cnt["dve"])
        # C state init: C = c/2 + 0.5 = 0.5 in u[(d,0)][:,32:40]
        for d in DIRS:
            nc.vector.memset(u[(d, 0)][:, 32:40], 0.5)
        for d in DIRS:
            pregate(l, d, 0)

        mm_done = {}
        sg_done = {}
        c_done = {}
        sc_done = {}
        h_done = {}

        for s in range(T):
            par = s % 2
            base = 32 * (s % 16)
            # ---- PE: rec matmuls fwd then bwd ----
            for d in DIRS:
                bank = gb[(d, (s // 16) % 2)]
                if s == 0:
                    rhs = zero8_ap()
                else:
                    W(nc.tensor, "dve", h_done[(d, s - 1)])
                    if d == "f":
                        rhs = Xout[:, (s - 1) * 8: s * 8]
                    else:
                        rhs = ap_of(Xout, NTOK + (T - s) * 8,
                                    [[pstride(Xout), 128], [1, 8]])
                last = None
                for g in range(4):
                    last = nc.tensor.matmul(
                        bank[:, base + 8 * g: base + 8 * g + 8],
                        whh_slice(l, d, g), rhs,
                        start=False, stop=True, skip_group_check=True)
                mm_done[(d, s)] = inc(last, "mm")
            # ---- PE: pregate pieces for chunk c+1 (safe: the target bank's
            # last sigma read was step 16c-1, ordered before via h-dep) ----
            j = s % 16
            if j == 0:
                pregate_bias(l, "f", s // 16 + 1)
            elif j == 1:
                pregate_part(l, "f", s // 16 + 1, 0)
            elif j == 2:
                pregate_part(l, "f", s // 16 + 1, 1)
            elif j == 3:
                pregate_bias(l, "b", s // 16 + 1)
            elif j == 4:
                pregate_part(l, "b", s // 16 + 1, 0)
            elif j == 5:
                pregate_part(l, "b", s // 16 + 1, 1)
            # ---- ACT: sigma over gates (both dirs) ----
            for d in DIRS:
                bank = gb[(d, (s // 16) % 2)]
                W(nc.scalar, "mm", mm_done[(d, s)])
                ins = nc.scalar.activation(u[(d, par)][:, 0:32],
                                           bank[:, base:base + 32], ActFn.Sigmoid)
                sg_done[(d, s)] = inc(ins, "act")
            # ---- DVE: [PQf, spacer, Cf, PQb, spacer, Cb] ----
            # The spacer (a) satisfies the DVE gap-0 RAW hazard between PQ and
            # C and (b) keeps Cf ahead of PQb's semaphore wait in the queue so
            # the fwd chain does not detour through the bwd gates.
            for d in DIRS:
                W(nc.vector, "act", sg_done[(d, s)])
                in0 = ap_of(u[(d, par)], 16, [[pstride(u[(d, par)]), 128], [16, 2], [1, 8]])
                in1 = ap_of(u[(d, par)], 0, [[pstride(u[(d, par)]), 128], [8, 2], [1, 8]])
                out = ap_of(pq[d], 0, [[pstride(pq[d]), 128], [8, 2], [1, 8]])
                nc.vector.scalar_tensor_tensor(out=out, in0=in0, scalar=0.5,
                                               in1=in1, op0=AluOp.subtract,
                                               op1=AluOp.mult)
                nc.vector.memset(dummy[:, :], 0.0)
                ins = nc.vector.scalar_tensor_tensor(
                    out=u[(d, 1 - par)][:, 32:40], in0=pq[d][:, 0:8], scalar=0.5,
                    in1=pq[d][:, 8:16], op0=AluOp.add, op1=AluOp.add)
                c_done[(d, s)] = inc(ins, "dve")
            # ---- ACT: v^ = sigma(4C-2) ----
            for d in DIRS:
                W(nc.scalar, "dve", c_done[(d, s)])
                ins = nc.scalar.activation(vhat[d][:, :], u[(d, 1 - par)][:, 32:40],
                                           ActFn.Sigmoid, scale=4.0, bias=-2.0)
                sc_done[(d, s)] = inc(ins, "act")
            # ---- DVE: h~ = (v^-0.5)*o~ ----
            for d in DIRS:
                W(nc.vector, "act", sc_done[(d, s)])
                if d == "f":
                    dst = Xout[:, s * 8:(s + 1) * 8]
                else:
                    dst = ap_of(Xout, NTOK + (T - 1 - s) * 8,
                                [[pstride(Xout), 128], [1, 8]])
                ins = nc.vector.scalar_tensor_tensor(
                    out=dst, in0=vhat[d][:, :], scalar=0.5,
                    in1=u[(d, par)][:, 24:32], op0=AluOp.subtract, op1=AluOp.mult)
                h_done[(d, s)] = inc(ins, "dve")

    layer(0, XL[1])
    layer(1, XL[2])

    # ---------------- FC (+ per-pair output DMA overlap) ----------------
    W(nc.tensor, "act", cnt["act"])
    W(nc.tensor, "dve", cnt["dve"])
    bfc_ap = ap_of(onesp, 512, [[pstride(onesp), 1], [1, 64]])
    ones_ap = ap_of(onesp, 0, [[pstride(onesp), 1], [1, 512]])
    wfca_ap = ap_of(fpack, 0, [[pstride(fpack), 128], [1, 64]])
    wfcb_ap = ap_of(fpack, 64, [[pstride(fpack), 128], [1, 64]])
    fc_copy = {}
    n_out = 0
    fc_starts = list(range(0, NTOK, 512))
    for i, st in enumerate(fc_starts):
        w = min(512, NTOK - st)
        bank = tbank[i % 4]
        if i >= 4:
            eng, c0 = fc_copy[i - 4]
            W(nc.tensor, eng, c0)
        nc.tensor.matmul(bank[0:64, 0:w], bfc_ap, ones_ap,
                         start=True, stop=False, skip_group_check=True)
        nc.tensor.matmul(bank[0:64, 0:w], wfca_ap, XL[2][:, st:st + w],
                         start=False, stop=False, skip_group_check=True)
        ins = nc.tensor.matmul(bank[0:64, 0:w], wfcb_ap,
                               ap_of(XL[2], NTOK + st, [[pstride(XL[2]), 128], [1, w]]),
                               start=False, stop=True, skip_group_check=True)
        mmc = inc(ins, "mm")
        if i % 2 == 0:
            W(nc.scalar, "mm", mmc)
            ins = nc.scalar.activation(y_s[:, st:st + w], bank[0:64, 0:w], ActFn.Copy)
            fc_copy[i] = ("act", inc(ins, "act"))
        else:
            W(nc.vector, "mm", mmc)
            ins = nc.vector.tensor_copy(y_s[:, st:st + w], bank[0:64, 0:w])
            fc_copy[i] = ("dve", inc(ins, "dve"))
        if i % 2 == 1:
            # DMA out the finished 1024-col slice while later chunks compute
            nc.sync.wait_ge(s_act, fc_copy[i - 1][1])
            nc.sync.wait_ge(s_dve, fc_copy[i][1])
            nc.sync.dma_start(out=y_d[:, st - 512:st + w],
                              in_=y_s[:, st - 512:st + w]).then_inc(s_out, 16)
            n_out += 1
    if len(fc_starts) % 2 == 1:
        st = fc_starts[-1]
        w = min(512, NTOK - st)
        nc.sync.wait_ge(s_act, cnt["act"])
        nc.sync.wait_ge(s_dve, cnt["dve"])
        nc.sync.dma_start(out=y_d[:, st:st + w],
                          in_=y_s[:, st:st + w]).then_inc(s_out, 16)
        n_out += 1

    nc.sync.wait_ge(s_out, 16 * n_out)
    return nc


# ====================== host-side prep & entry point ======================

def _to_bf(a):
    return np.asarray(a, dtype=np.float32).astype(BF)


def prep_weights(inputs):
    """Pre-scaled lhsT tensors per the v2 formulation."""
    out = {}
    for l in (0, 1):
        xin_scale = 1.0 if l == 0 else 2.0
        for dname, suf in (("f", ""), ("b", "r")):
            wih = np.asarray(inputs[f"w_ih_l{l}{suf}"], np.float32)   # [512, Din]
            whh = np.asarray(inputs[f"w_hh_l{l}{suf}"], np.float32)   # [512, 128]
            bsum = (np.asarray(inputs[f"b_ih_l{l}{suf}"], np.float32)
                    + np.asarray(inputs[f"b_hh_l{l}{suf}"], np.float32))
            blk_ih, blk_hh, b4 = [], [], np.zeros((4, 128), np.float32)
            for gi, G in enumerate(GO):
                rows = slice(PT[G] * 128, (PT[G] + 1) * 128)
                sG = 2.0 if G == "g" else 1.0
                blk_ih.append((sG * xin_scale * wih[rows]).T)   # [Din,128]
                blk_hh.append((sG * 2.0 * whh[rows]).T)         # [128,128]
                b4[gi] = sG * bsum[rows]
            wih_cat = np.concatenate(blk_ih, axis=1)            # [Din, 512]
            out[f"whh{l}{dname}"] = _to_bf(np.concatenate(blk_hh, axis=1))
            out[f"b4_{l}{dname}"] = _to_bf(b4)
            if l == 0:
                out[f"wih0{dname}"] = _to_bf(wih_cat)           # [64, 512]
            else:
                out[f"wih1a{dname}"] = _to_bf(wih_cat[0:128])
                out[f"wih1b{dname}"] = _to_bf(wih_cat[128:256])
    wfc = 2.0 * np.asarray(inputs["w_fc"], np.float32).T        # [256, 64]
    out["wfca"] = _to_bf(wfc[0:128])
    out["wfcb"] = _to_bf(wfc[128:256])
    out["bfc"] = np.asarray(inputs["b_fc"], np.float32).reshape(1, 64)
    return out


def _mask4_np():
    m = np.zeros((4, 512), np.float32)
    for g in range(4):
        for r in range(16):
            m[g, r * 32 + g * 8: r * 32 + g * 8 + 8] = 1.0
    return m.astype(BF)


_NC_CACHE = {}


def _get_nc(T):
    if T not in _NC_CACHE:
        _NC_CACHE[T] = build_nc(T)
    return _NC_CACHE[T]


def run_cores(inputs, T=512, n_cores=8, trace=False):
    x = np.asarray(inputs["x"], np.float32)
    per = 8
    wp = prep_weights(inputs)
    wpack = np.concatenate([wp["whh0f"], wp["whh0b"], wp["whh1f"], wp["whh1b"],
                            wp["wih1af"], wp["wih1ab"], wp["wih1bf"], wp["wih1bb"]],
                           axis=1)                                    # [128, 4096]
    wih0p = np.concatenate([wp["wih0f"], wp["wih0b"]], axis=1)        # [64, 1024]
    bpack = np.concatenate([wp["b4_0f"], wp["b4_0b"], wp["b4_1f"], wp["b4_1b"],
                            _mask4_np()], axis=1)                     # [4, 1024]
    fpack = np.concatenate([wp["wfca"], wp["wfcb"],
                            np.zeros((128, 8), BF)], axis=1)          # [128, 136]
    onesp = np.concatenate([np.ones((1, 512), np.float32),
                            wp["bfc"]], axis=1)                       # [1, 576]
    common = {
        "wpack": wpack, "wih0p": wih0p, "bpack": bpack, "fpack": fpack,
        "onesp": onesp, "id128_in": np.eye(128, dtype=np.float32),
    }
    in_maps = []
    for c in range(n_cores):
        m = dict(common)
        xc = x[c * per:(c + 1) * per, :T]                     # [8, T, 64]
        m["x0"] = np.ascontiguousarray(
            xc.transpose(2, 1, 0).reshape(64, T * 8)).astype(BF)
        in_maps.append(m)

    nc = _get_nc(T)
    res = run_bass_kernel_spmd(nc, in_maps, core_ids=list(range(n_cores)), trace=trace)
    outs = []
    for c in range(n_cores):
        yc = res.results[c]["y"]
        outs.append(yc.reshape(64, T, 8).transpose(2, 1, 0))
    return np.concatenate(outs, axis=0), res


def kernel(**inputs):
    y, _ = run_cores(inputs, T=512, n_cores=8)
    return y.astype(np.float32)


# revision 8
# speedup vs baseline: 1240.9379x; 1.0092x over previous
"""Trainium2 Bass kernel for nn_BiLSTM: 2-layer BiLSTM (B=64,T=512,D=64,H=128) + FC.

Sharding: data-parallel over batch across 8 NeuronCores (8 samples/core).

v2 design (split-dir, in-phase lockstep, min-chain cell update):
  Per layer, fwd and bwd run as two independent recurrent chains advanced in
  lockstep; per step s each dir does:
    PE : 4 gate matmuls  gates += Whh_g @ h~(s-1)    [128,8] into PSUM bank
    ACT: sg = Sigmoid(bank[32 cols])          -> u[par][0:32]   (i,f,g,o x 8)
    DVE: PQ = (in0 - .5) * in1  where in0=[g~|C], in1=[i~|f~]   -> [p|q]
    DVE: C' = (p + .5) + q                    -> u[par^1][32:40]
    ACT: v^ = Sigmoid(4C' - 2)                -> vhat
    DVE: h~ = (v^ - .5) * o~                  -> X[l] column (bf16)
  State: C = c/2 + 0.5 kept adjacent to the sigma outputs so PQ is one
  strided-AP op.  h~ = h/2; consumers (Whh, Wih_l1, Wfc) pre-scaled by 2.
  g-gate rows pre-scaled by 2 so one Sigmoid covers tanh(g) via 2sig(2g)-1.

  Engine queue order per step: PE [mmf x4, mmb x4, pregate burst]
                               ACT [sgf, sgb, scf, scb]
                               DVE [PQf, PQb, Cf, Cb, hf, hb]
  Pregates (x-part + bias) accumulate into 2 PSUM banks/dir (16 steps each),
  staged 2 chunks ahead during PE idle; no explicit waits needed (transitively
  ordered through the h-dependency).
Host: reshape y -> [8,T,64] per core, concat cores -> [64,T,64].
"""
import sys
sys.path.insert(0, "/opt/trn_rl_repo")
import numpy as np
import ml_dtypes

import concourse.bass as bass
from concourse import mybir
from concourse.bass_utils import run_bass_kernel_spmd

F32 = mybir.dt.float32
BF16 = mybir.dt.bfloat16
BF = ml_dtypes.bfloat16
AluOp = mybir.AluOpType
ActFn = mybir.ActivationFunctionType

H = 128
PT = {"i": 0, "f": 1, "g": 2, "o": 3}   # PyTorch row-block order
GO = ["i", "f", "g", "o"]               # PSUM/u col-block order (8 cols each)
DIRS = ("f", "b")


def ap_of(t, off, dims):
    base = t[:] if not isinstance(t, bass.AP) else t
    return bass.AP(tensor=base.tensor, offset=base.offset + off, ap=list(dims))


def pstride(t):
    base = t[:] if not isinstance(t, bass.AP) else t
    return base.ap[0][0]


def build_nc(T=512):
    assert T % 16 == 0
    NTOK = T * 8
    NCH = T // 16                      # pregate chunks (16 steps each)
    nc = bass.Bass("TRN2", target_bir_lowering=False, debug=False)

    # register -2.0 const AP (sigma-cell bias), same pattern as Bass.__init__
    _c = nc.alloc_sbuf_tensor("const-f32-neg2", [128, 1], F32)
    nc.gpsimd.memset(_c.ap(), -2.0)
    nc.const_aps.aps[(F32, -2.0)] = _c.ap()
    nc.all_engine_barrier()

    # ---------------- DRAM I/O (batched packs to cut DMA descriptors) ----
    # x0: host-pretransposed input  [64 d, t*8+b]  bf16
    x0_d = nc.dram_tensor("x0", [64, NTOK], BF16, kind="ExternalInput")
    # wpack cols: whh0f whh0b whh1f whh1b wih1af wih1ab wih1bf wih1bb (x512)
    wpack_d = nc.dram_tensor("wpack", [128, 4096], BF16, kind="ExternalInput")
    wih0p_d = nc.dram_tensor("wih0p", [64, 1024], BF16, kind="ExternalInput")
    # bpack cols: b4(0f) b4(0b) b4(1f) b4(1b) (x128), mask4 (512)
    bpack_d = nc.dram_tensor("bpack", [4, 1024], BF16, kind="ExternalInput")
    # fpack cols: wfca(64) wfcb(64) zero8(8)
    fpack_d = nc.dram_tensor("fpack", [128, 136], BF16, kind="ExternalInput")
    bfc64_d = nc.dram_tensor("bfc64", [64, 1], F32, kind="ExternalInput")
    y_d = nc.dram_tensor("y", [64, NTOK], F32, kind="ExternalOutput")

    # ---------------- SBUF ----------------
    sb = nc.alloc_sbuf_tensor
    X0 = sb("X0", [64, NTOK], BF16)
    XL = {1: sb("XL1", [128, 2 * NTOK], BF16), 2: sb("XL2", [128, 2 * NTOK], BF16)}
    y_s = sb("y_s", [64, NTOK], F32)

    wpack = sb("wpack_s", [128, 4096], BF16)
    wih0p = sb("wih0p_s", [64, 1024], BF16)
    bpack = sb("bpack_s", [4, 1024], BF16)
    fpack = sb("fpack_s", [128, 136], BF16)
    bfc64 = sb("bfc64_s", [64, 1], F32)

    WOFF = {(0, "f"): 0, (0, "b"): 512, (1, "f"): 1024, (1, "b"): 1536}
    W1A = {"f": 2048, "b": 2560}
    W1B = {"f": 3072, "b": 3584}
    B4OFF = {(0, "f"): 0, (0, "b"): 128, (1, "f"): 256, (1, "b"): 384}

    def whh_slice(l, d, g):
        return ap_of(wpack, WOFF[(l, d)] + g * 128, [[pstride(wpack), 128], [1, 128]])

    def zero8_ap():
        return ap_of(fpack, 128, [[pstride(fpack), 128], [1, 8]])

    # u[(d,par)]: cols 0:32 = sigma(gates) [i f g o]; cols 32:40 = C state
    u = {(d, p): sb(f"u_{d}{p}", [128, 40], F32) for d in DIRS for p in (0, 1)}
    pq = {d: sb(f"pq_{d}", [128, 16], F32) for d in DIRS}
    vhat = {d: sb(f"vhat_{d}", [128, 8], F32) for d in DIRS}
    dummy = sb("dummy_sp", [128, 1], F32)

    gb = {(d, i): nc.alloc_psum_tensor(f"gb_{d}{i}", [128, 512], F32)
          for d in DIRS for i in (0, 1)}
    tbank = [nc.alloc_psum_tensor(f"tb{i}", [64, 512], F32) for i in range(4)]

    sem_in = nc.alloc_semaphore("sem_in")
    s_mm = nc.alloc_semaphore("s_mm")
    s_act = nc.alloc_semaphore("s_act")
    s_dve = nc.alloc_semaphore("s_dve")
    s_out = nc.alloc_semaphore("s_out")
    cnt = {"mm": 0, "act": 0, "dve": 0}
    sems = {"mm": s_mm, "act": s_act, "dve": s_dve}

    def W(eng, which, val):
        eng.wait_ge(sems[which], val)

    # pre-warm the sigmoid activation table set while input DMAs stream
    # (the first ACT instruction otherwise pays the ~2.7us table load on the
    # critical path; Copy lives in every set so it won't trigger a reload)
    nc.scalar.activation(dummy[:, :], dummy[:, :], ActFn.Sigmoid)

    def inc(ins, which):
        ins.then_inc(sems[which], 1)
        cnt[which] += 1
        return cnt[which]

    # ---------------- input DMAs ----------------
    n_dma = 0

    def dma(dst, src):
        nonlocal n_dma
        nc.sync.dma_start(out=dst, in_=src).then_inc(sem_in, 16)
        n_dma += 1

    # spread the two big DMAs across different engines' queues so the
    # transfers run on parallel rings instead of serializing on one
    dma(X0[:, :], x0_d[:, :])
    nc.gpsimd.dma_start(out=wpack[:, :], in_=wpack_d[:, :]).then_inc(sem_in, 16)
    n_dma += 1
    dma(wih0p[:, :], wih0p_d[:, :])
    dma(bpack[:, :], bpack_d[:, :])
    dma(fpack[:, :], fpack_d[:, :])
    dma(bfc64[:, :], bfc64_d[:, :])

    # ---------------- BiLSTM layers ----------------
    def pregate_bias(l, d, c):
        if c >= NCH:
            return
        bank = gb[(d, c % 2)]
        b4ap = ap_of(bpack, B4OFF[(l, d)], [[pstride(bpack), 4], [1, 128]])
        mask4ap = ap_of(bpack, 512, [[pstride(bpack), 4], [1, 512]])
        nc.tensor.matmul(bank[:, 0:512], b4ap, mask4ap,
                         start=True, stop=False, skip_group_check=True)

    def pregate_part(l, d, c, pi):
        """x-part matmuls (part pi) for chunk c (steps 16c..16c+15) of dir d."""
        if c >= NCH:
            return
        bank = gb[(d, c % 2)]
        if l == 0:
            parts = [(wih0p, 0 if d == "f" else 512, X0, 64, 0)]
        else:
            parts = [(wpack, W1A[d], XL[1], 128, 0),
                     (wpack, W1B[d], XL[1], 128, NTOK)]
        if pi >= len(parts):
            return
        (wt, woff, Xsrc, K, xoff) = parts[pi]
        if d == "f":
            rhs = ap_of(Xsrc, xoff + c * 128, [[pstride(Xsrc), K], [1, 128]])
        else:
            # step j of chunk c handles time T-1-16c-j  -> negative stride
            rhs = ap_of(Xsrc, xoff + (T - 1 - 16 * c) * 8,
                        [[pstride(Xsrc), K], [-8, 16], [1, 8]])
        for g in range(4):
            dst = ap_of(bank, 8 * g, [[pstride(bank), 128], [32, 16], [1, 8]])
            lhsT = ap_of(wt, woff + g * 128, [[pstride(wt), K], [1, 128]])
            nc.tensor.matmul(dst, lhsT, rhs,
                             start=False, stop=False, skip_group_check=True)

    def pregate(l, d, c):
        pregate_bias(l, d, c)
        pregate_part(l, d, c, 0)
        pregate_part(l, d, c, 1)

    def layer(l, Xout):
        # barrier: inputs (X0 or XL1) fully written; weights DMA'd
        if l == 0:
            nc.tensor.wait_ge(sem_in, 16 * n_dma)
        W(nc.tensor, "act", cnt["act"])
        W(nc.tensor, "dve", cnt["dve"])
        # C state init: C = c/2 + 0.5 = 0.5 in u[(d,0)][:,32:40]
        for d in DIRS:
            nc.vector.memset(u[(d, 0)][:, 32:40], 0.5)
        for d in DIRS:
            pregate(l, d, 0)

        mm_done = {}
        sg_done = {}
        c_done = {}
        sc_done = {}
        h_done = {}

        for s in range(T):
            par = s % 2
            base = 32 * (s % 16)
            # ---- PE: rec matmuls fwd then bwd ----
            for d in DIRS:
                bank = gb[(d, (s // 16) % 2)]
                if s == 0:
                    rhs = zero8_ap()
                else:
                    W(nc.tensor, "dve", h_done[(d, s - 1)])
                    if d == "f":
                        rhs = Xout[:, (s - 1) * 8: s * 8]
                    else:
                        rhs = ap_of(Xout, NTOK + (T - s) * 8,
                                    [[pstride(Xout), 128], [1, 8]])
                last = None
                for g in range(4):
                    last = nc.tensor.matmul(
                        bank[:, base + 8 * g: base + 8 * g + 8],
                        whh_slice(l, d, g), rhs,
                        start=False, stop=True, skip_group_check=True)
                mm_done[(d, s)] = inc(last, "mm")
            # ---- PE: pregate pieces for chunk c+1 (safe: the target bank's
            # last sigma read was step 16c-1, ordered before via h-dep) ----
            j = s % 16
            if j == 0:
                pregate_bias(l, "f", s // 16 + 1)
            elif j == 1:
                pregate_part(l, "f", s // 16 + 1, 0)
            elif j == 2:
                pregate_part(l, "f", s // 16 + 1, 1)
            elif j == 3:
                pregate_bias(l, "b", s // 16 + 1)
            elif j == 4:
                pregate_part(l, "b", s // 16 + 1, 0)
            elif j == 5:
                pregate_part(l, "b", s // 16 + 1, 1)
            # ---- ACT: sigma over gates (both dirs) ----
            for d in DIRS:
                bank = gb[(d, (s // 16) % 2)]
                W(nc.scalar, "mm", mm_done[(d, s)])
                ins = nc.scalar.activation(u[(d, par)][:, 0:32],
                                           bank[:, base:base + 32], ActFn.Sigmoid)
                sg_done[(d, s)] = inc(ins, "act")
            # ---- DVE: [PQf, spacer, Cf, PQb, spacer, Cb] ----
            # The spacer (a) satisfies the DVE gap-0 RAW hazard between PQ and
            # C and (b) keeps Cf ahead of PQb's semaphore wait in the queue so
            # the fwd chain does not detour through the bwd gates.
            for d in DIRS:
                W(nc.vector, "act", sg_done[(d, s)])
                in0 = ap_of(u[(d, par)], 16, [[pstride(u[(d, par)]), 128], [16, 2], [1, 8]])
                in1 = ap_of(u[(d, par)], 0, [[pstride(u[(d, par)]), 128], [8, 2], [1, 8]])
                out = ap_of(pq[d], 0, [[pstride(pq[d]), 128], [8, 2], [1, 8]])
                nc.vector.scalar_tensor_tensor(out=out, in0=in0, scalar=0.5,
                                               in1=in1, op0=AluOp.subtract,
                                               op1=AluOp.mult)
                nc.vector.memset(dummy[:, :], 0.0)
                ins = nc.vector.scalar_tensor_tensor(
                    out=u[(d, 1 - par)][:, 32:40], in0=pq[d][:, 0:8], scalar=0.5,
                    in1=pq[d][:, 8:16], op0=AluOp.add, op1=AluOp.add)
                c_done[(d, s)] = inc(ins, "dve")
            # ---- ACT: v^ = sigma(4C-2) ----
            for d in DIRS:
                W(nc.scalar, "dve", c_done[(d, s)])
                ins = nc.scalar.activation(vhat[d][:, :], u[(d, 1 - par)][:, 32:40],
                                           ActFn.Sigmoid, scale=4.0, bias=-2.0)
                sc_done[(d, s)] = inc(ins, "act")
            # ---- DVE: h~ = (v^-0.5)*o~ ----
            for d in DIRS:
                W(nc.vector, "act", sc_done[(d, s)])
                if d == "f":
                    dst = Xout[:, s * 8:(s + 1) * 8]
                else:
                    dst = ap_of(Xout, NTOK + (T - 1 - s) * 8,
                                [[pstride(Xout), 128], [1, 8]])
                ins = nc.vector.scalar_tensor_tensor(
                    out=dst, in0=vhat[d][:, :], scalar=0.5,
                    in1=u[(d, par)][:, 24:32], op0=AluOp.subtract, op1=AluOp.mult)
                h_done[(d, s)] = inc(ins, "dve")

    layer(0, XL[1])
    layer(1, XL[2])

    # ---------------- FC (+ per-pair output DMA overlap) ----------------
    # y = 2*Wfc @ [X2f; X2b] (+bias): two bf16 matmuls per 512-col chunk; the
    # per-row bias rides the PSUM->SBUF copy as an ACT Identity bias AP.
    W(nc.tensor, "act", cnt["act"])
    W(nc.tensor, "dve", cnt["dve"])
    wfca_ap = ap_of(fpack, 0, [[pstride(fpack), 128], [1, 64]])
    wfcb_ap = ap_of(fpack, 64, [[pstride(fpack), 128], [1, 64]])
    bias_ap = ap_of(bfc64, 0, [[pstride(bfc64), 64], [1, 1]])
    fc_copy = {}
    n_out = 0
    fc_starts = list(range(0, NTOK, 512))
    for i, st in enumerate(fc_starts):
        w = min(512, NTOK - st)
        bank = tbank[i % 4]
        if i >= 4:
            eng, c0 = fc_copy[i - 4]
            W(nc.tensor, eng, c0)
        nc.tensor.matmul(bank[0:64, 0:w], wfca_ap, XL[2][:, st:st + w],
                         start=True, stop=False, skip_group_check=True)
        ins = nc.tensor.matmul(bank[0:64, 0:w], wfcb_ap,
                               ap_of(XL[2], NTOK + st, [[pstride(XL[2]), 128], [1, w]]),
                               start=False, stop=True, skip_group_check=True)
        mmc = inc(ins, "mm")
        W(nc.scalar, "mm", mmc)
        ins = nc.scalar.activation(y_s[:, st:st + w], bank[0:64, 0:w],
                                   ActFn.Identity, bias=bias_ap)
        fc_copy[i] = ("act", inc(ins, "act"))
        if i % 2 == 1:
            # DMA out the finished 1024-col slice while later chunks compute
            nc.sync.wait_ge(s_act, fc_copy[i][1])
            nc.sync.dma_start(out=y_d[:, st - 512:st + w],
                              in_=y_s[:, st - 512:st + w]).then_inc(s_out, 16)
            n_out += 1
    if len(fc_starts) % 2 == 1:
        st = fc_starts[-1]
        w = min(512, NTOK - st)
        nc.sync.wait_ge(s_act, cnt["act"])
        nc.sync.dma_start(out=y_d[:, st:st + w],
                          in_=y_s[:, st:st + w]).then_inc(s_out, 16)
        n_out += 1

    nc.sync.wait_ge(s_out, 16 * n_out)
    return nc


# ====================== host-side prep & entry point ======================

def _to_bf(a):
    return np.asarray(a, dtype=np.float32).astype(BF)


def prep_weights(inputs):
    """Pre-scaled lhsT tensors per the v2 formulation."""
    out = {}
    for l in (0, 1):
        xin_scale = 1.0 if l == 0 else 2.0
        for dname, suf in (("f", ""), ("b", "r")):
            wih = np.asarray(inputs[f"w_ih_l{l}{suf}"], np.float32)   # [512, Din]
            whh = np.asarray(inputs[f"w_hh_l{l}{suf}"], np.float32)   # [512, 128]
            bsum = (np.asarray(inputs[f"b_ih_l{l}{suf}"], np.float32)
                    + np.asarray(inputs[f"b_hh_l{l}{suf}"], np.float32))
            blk_ih, blk_hh, b4 = [], [], np.zeros((4, 128), np.float32)
            for gi, G in enumerate(GO):
                rows = slice(PT[G] * 128, (PT[G] + 1) * 128)
                sG = 2.0 if G == "g" else 1.0
                blk_ih.append((sG * xin_scale * wih[rows]).T)   # [Din,128]
                blk_hh.append((sG * 2.0 * whh[rows]).T)         # [128,128]
                b4[gi] = sG * bsum[rows]
            wih_cat = np.concatenate(blk_ih, axis=1)            # [Din, 512]
            out[f"whh{l}{dname}"] = _to_bf(np.concatenate(blk_hh, axis=1))
            out[f"b4_{l}{dname}"] = _to_bf(b4)
            if l == 0:
                out[f"wih0{dname}"] = _to_bf(wih_cat)           # [64, 512]
            else:
                out[f"wih1a{dname}"] = _to_bf(wih_cat[0:128])
                out[f"wih1b{dname}"] = _to_bf(wih_cat[128:256])
    wfc = 2.0 * np.asarray(inputs["w_fc"], np.float32).T        # [256, 64]
    out["wfca"] = _to_bf(wfc[0:128])
    out["wfcb"] = _to_bf(wfc[128:256])
    out["bfc"] = np.asarray(inputs["b_fc"], np.float32).reshape(1, 64)
    return out


def _mask4_np():
    m = np.zeros((4, 512), np.float32)
    for g in range(4):
        for r in range(16):
            m[g, r * 32 + g * 8: r * 32 + g * 8 + 8] = 1.0
    return m.astype(BF)


_NC_CACHE = {}


def _get_nc(T):
    if T not in _NC_CACHE:
        _NC_CACHE[T] = build_nc(T)
    return _NC_CACHE[T]


def run_cores(inputs, T=512, n_cores=8, trace=False):
    x = np.asarray(inputs["x"], np.float32)
    per = 8
    wp = prep_weights(inputs)
    wpack = np.concatenate([wp["whh0f"], wp["whh0b"], wp["whh1f"], wp["whh1b"],
                            wp["wih1af"], wp["wih1ab"], wp["wih1bf"], wp["wih1bb"]],
                           axis=1)                                    # [128, 4096]
    wih0p = np.concatenate([wp["wih0f"], wp["wih0b"]], axis=1)        # [64, 1024]
    bpack = np.concatenate([wp["b4_0f"], wp["b4_0b"], wp["b4_1f"], wp["b4_1b"],
                            _mask4_np()], axis=1)                     # [4, 1024]
    fpack = np.concatenate([wp["wfca"], wp["wfcb"],
                            np.zeros((128, 8), BF)], axis=1)          # [128, 136]
    common = {
        "wpack": wpack, "wih0p": wih0p, "bpack": bpack, "fpack": fpack,
        "bfc64": wp["bfc"].reshape(64, 1),
    }
    in_maps = []
    for c in range(n_cores):
        m = dict(common)
        xc = x[c * per:(c + 1) * per, :T]                     # [8, T, 64]
        m["x0"] = np.ascontiguousarray(
            xc.transpose(2, 1, 0).reshape(64, T * 8)).astype(BF)
        in_maps.append(m)

    nc = _get_nc(T)
    res = run_bass_kernel_spmd(nc, in_maps, core_ids=list(range(n_cores)), trace=trace)
    outs = []
    for c in range(n_cores):
        yc = res.results[c]["y"]
        outs.append(yc.reshape(64, T, 8).transpose(2, 1, 0))
    return np.concatenate(outs, axis=0), res


def kernel(**inputs):
    y, _ = run_cores(inputs, T=512, n_cores=8)
    return y.astype(np.float32)


# revision 9
# speedup vs baseline: 1240.9861x; 1.0000x over previous
"""Trainium2 Bass kernel for nn_BiLSTM: 2-layer BiLSTM (B=64,T=512,D=64,H=128) + FC.

Sharding: data-parallel over batch across 8 NeuronCores (8 samples/core).

Design (split-dir, in-phase lockstep, minimal-chain LSTM cell, ~1867ns/step):
  Per layer, fwd and bwd run as two independent recurrent chains advanced in
  lockstep; per step s each dir does:
    PE : 4 gate matmuls  gates += Whh_g @ h~(s-1)    [128,8] into PSUM bank
    ACT: sg = Sigmoid(bank[32 cols])          -> u[par][0:32]   (i,f,g,o x 8)
    DVE: PQ = (in0 - .5) * in1  where in0=[g~|C], in1=[i~|f~]   -> [p|q]
    DVE: C' = (p + .5) + q                    -> u[par^1][32:40]
    ACT: v^ = Sigmoid(4C' - 2)                -> vhat
    DVE: h~ = (v^ - .5) * o~                  -> X[l] column (bf16)
  State: C = c/2 + 0.5 kept adjacent to the sigma outputs so PQ is one
  strided-AP op.  h~ = h/2; consumers (Whh, Wih_l1, Wfc) pre-scaled by 2.
  g-gate rows pre-scaled by 2 so one Sigmoid covers tanh(g) via 2sig(2g)-1.

  Engine queue order per step: PE [mmf x4, mmb x4, pregate piece]
                               ACT [sgf, sgb, scf, scb]
                               DVE [PQf, ms, Cf, PQb, ms, Cb, hf, hb]
  The memset spacers satisfy the DVE gap-0 RAW hazard AND keep Cf from
  queueing behind PQb's semaphore wait (fwd chain must not detour through
  the bwd gates).  Pregates (bias+x-part) accumulate into 2 PSUM banks/dir
  (16 steps each), staged 1 chunk ahead in small pieces during PE idle; no
  explicit waits needed (transitively ordered through the h-dependency).
  x is transposed to [64d, t*8+b] bf16 on the host; FC bias rides the
  PSUM->SBUF copy as an ACT Identity per-partition bias AP; outputs DMA out
  per 1024-col slice overlapping later FC chunks.
Host: reshape y -> [8,T,64] per core, concat cores -> [64,T,64].
"""
import sys
sys.path.insert(0, "/opt/trn_rl_repo")
import numpy as np
import ml_dtypes

import concourse.bass as bass
from concourse import mybir
from concourse.bass_utils import run_bass_kernel_spmd

F32 = mybir.dt.float32
BF16 = mybir.dt.bfloat16
BF = ml_dtypes.bfloat16
AluOp = mybir.AluOpType
ActFn = mybir.ActivationFunctionType

H = 128
PT = {"i": 0, "f": 1, "g": 2, "o": 3}   # PyTorch row-block order
GO = ["i", "f", "g", "o"]               # PSUM/u col-block order (8 cols each)
DIRS = ("f", "b")


def ap_of(t, off, dims):
    base = t[:] if not isinstance(t, bass.AP) else t
    return bass.AP(tensor=base.tensor, offset=base.offset + off, ap=list(dims))


def pstride(t):
    base = t[:] if not isinstance(t, bass.AP) else t
    return base.ap[0][0]


def build_nc(T=512):
    assert T % 16 == 0
    NTOK = T * 8
    NCH = T // 16                      # pregate chunks (16 steps each)
    nc = bass.Bass("TRN2", target_bir_lowering=False, debug=False)

    # register -2.0 const AP (sigma-cell bias), same pattern as Bass.__init__
    _c = nc.alloc_sbuf_tensor("const-f32-neg2", [128, 1], F32)
    nc.gpsimd.memset(_c.ap(), -2.0)
    nc.const_aps.aps[(F32, -2.0)] = _c.ap()
    nc.all_engine_barrier()

    # ---------------- DRAM I/O (batched packs to cut DMA descriptors) ----
    # x0: host-pretransposed input  [64 d, t*8+b]  bf16
    x0_d = nc.dram_tensor("x0", [64, NTOK], BF16, kind="ExternalInput")
    # wpack cols: whh0f whh0b whh1f whh1b wih1af wih1ab wih1bf wih1bb (x512)
    wpack_d = nc.dram_tensor("wpack", [128, 4096], BF16, kind="ExternalInput")
    wih0p_d = nc.dram_tensor("wih0p", [64, 1024], BF16, kind="ExternalInput")
    # bpack cols: b4(0f) b4(0b) b4(1f) b4(1b) (x128), mask4 (512)
    bpack_d = nc.dram_tensor("bpack", [4, 1024], BF16, kind="ExternalInput")
    # fpack cols: wfca(64) wfcb(64) zero8(8)
    fpack_d = nc.dram_tensor("fpack", [128, 136], BF16, kind="ExternalInput")
    bfc64_d = nc.dram_tensor("bfc64", [64, 1], F32, kind="ExternalInput")
    y_d = nc.dram_tensor("y", [64, NTOK], F32, kind="ExternalOutput")

    # ---------------- SBUF ----------------
    sb = nc.alloc_sbuf_tensor
    X0 = sb("X0", [64, NTOK], BF16)
    XL = {1: sb("XL1", [128, 2 * NTOK], BF16), 2: sb("XL2", [128, 2 * NTOK], BF16)}
    y_s = sb("y_s", [64, NTOK], F32)

    wpack = sb("wpack_s", [128, 4096], BF16)
    wih0p = sb("wih0p_s", [64, 1024], BF16)
    bpack = sb("bpack_s", [4, 1024], BF16)
    fpack = sb("fpack_s", [128, 136], BF16)
    bfc64 = sb("bfc64_s", [64, 1], F32)

    WOFF = {(0, "f"): 0, (0, "b"): 512, (1, "f"): 1024, (1, "b"): 1536}
    W1A = {"f": 2048, "b": 2560}
    W1B = {"f": 3072, "b": 3584}
    B4OFF = {(0, "f"): 0, (0, "b"): 128, (1, "f"): 256, (1, "b"): 384}

    def whh_slice(l, d, g):
        return ap_of(wpack, WOFF[(l, d)] + g * 128, [[pstride(wpack), 128], [1, 128]])

    def zero8_ap():
        return ap_of(fpack, 128, [[pstride(fpack), 128], [1, 8]])

    # u[(d,par)]: cols 0:32 = sigma(gates) [i f g o]; cols 32:40 = C state
    u = {(d, p): sb(f"u_{d}{p}", [128, 40], F32) for d in DIRS for p in (0, 1)}
    pq = {d: sb(f"pq_{d}", [128, 16], F32) for d in DIRS}
    vhat = {d: sb(f"vhat_{d}", [128, 8], F32) for d in DIRS}
    dummy = sb("dummy_sp", [128, 1], F32)

    gb = {(d, i): nc.alloc_psum_tensor(f"gb_{d}{i}", [128, 512], F32)
          for d in DIRS for i in (0, 1)}
    tbank = [nc.alloc_psum_tensor(f"tb{i}", [64, 512], F32) for i in range(4)]

    sem_in = nc.alloc_semaphore("sem_in")
    s_mm = nc.alloc_semaphore("s_mm")
    s_act = nc.alloc_semaphore("s_act")
    s_dve = nc.alloc_semaphore("s_dve")
    s_out = nc.alloc_semaphore("s_out")
    cnt = {"mm": 0, "act": 0, "dve": 0}
    sems = {"mm": s_mm, "act": s_act, "dve": s_dve}

    def W(eng, which, val):
        eng.wait_ge(sems[which], val)

    # pre-warm the sigmoid activation table set while input DMAs stream
    # (the first ACT instruction otherwise pays the ~2.7us table load on the
    # critical path; Copy lives in every set so it won't trigger a reload)
    nc.scalar.activation(dummy[:, :], dummy[:, :], ActFn.Sigmoid)

    def inc(ins, which):
        ins.then_inc(sems[which], 1)
        cnt[which] += 1
        return cnt[which]

    # ---------------- input DMAs ----------------
    n_dma = 0

    def dma(dst, src):
        nonlocal n_dma
        nc.sync.dma_start(out=dst, in_=src).then_inc(sem_in, 16)
        n_dma += 1

    # spread the two big DMAs across different engines' queues so the
    # transfers run on parallel rings instead of serializing on one
    dma(X0[:, :], x0_d[:, :])
    nc.gpsimd.dma_start(out=wpack[:, :], in_=wpack_d[:, :]).then_inc(sem_in, 16)
    n_dma += 1
    dma(wih0p[:, :], wih0p_d[:, :])
    dma(bpack[:, :], bpack_d[:, :])
    dma(fpack[:, :], fpack_d[:, :])
    dma(bfc64[:, :], bfc64_d[:, :])

    # ---------------- BiLSTM layers ----------------
    def pregate_bias(l, d, c):
        if c >= NCH:
            return
        bank = gb[(d, c % 2)]
        b4ap = ap_of(bpack, B4OFF[(l, d)], [[pstride(bpack), 4], [1, 128]])
        mask4ap = ap_of(bpack, 512, [[pstride(bpack), 4], [1, 512]])
        nc.tensor.matmul(bank[:, 0:512], b4ap, mask4ap,
                         start=True, stop=False, skip_group_check=True)

    def pregate_part(l, d, c, pi):
        """x-part matmuls (part pi) for chunk c (steps 16c..16c+15) of dir d."""
        if c >= NCH:
            return
        bank = gb[(d, c % 2)]
        if l == 0:
            parts = [(wih0p, 0 if d == "f" else 512, X0, 64, 0)]
        else:
            parts = [(wpack, W1A[d], XL[1], 128, 0),
                     (wpack, W1B[d], XL[1], 128, NTOK)]
        if pi >= len(parts):
            return
        (wt, woff, Xsrc, K, xoff) = parts[pi]
        if d == "f":
            rhs = ap_of(Xsrc, xoff + c * 128, [[pstride(Xsrc), K], [1, 128]])
        else:
            # step j of chunk c handles time T-1-16c-j  -> negative stride
            rhs = ap_of(Xsrc, xoff + (T - 1 - 16 * c) * 8,
                        [[pstride(Xsrc), K], [-8, 16], [1, 8]])
        for g in range(4):
            dst = ap_of(bank, 8 * g, [[pstride(bank), 128], [32, 16], [1, 8]])
            lhsT = ap_of(wt, woff + g * 128, [[pstride(wt), K], [1, 128]])
            nc.tensor.matmul(dst, lhsT, rhs,
                             start=False, stop=False, skip_group_check=True)

    def pregate(l, d, c):
        pregate_bias(l, d, c)
        pregate_part(l, d, c, 0)
        pregate_part(l, d, c, 1)

    def layer(l, Xout):
        # barrier: inputs (X0 or XL1) fully written; weights DMA'd
        if l == 0:
            nc.tensor.wait_ge(sem_in, 16 * n_dma)
        W(nc.tensor, "act", cnt["act"])
        W(nc.tensor, "dve", cnt["dve"])
        # C state init: C = c/2 + 0.5 = 0.5 in u[(d,0)][:,32:40]
        for d in DIRS:
            nc.vector.memset(u[(d, 0)][:, 32:40], 0.5)
        for d in DIRS:
            pregate(l, d, 0)

        mm_done = {}
        sg_done = {}
        c_done = {}
        sc_done = {}
        h_done = {}

        for s in range(T):
            par = s % 2
            base = 32 * (s % 16)
            # ---- PE: rec matmuls fwd then bwd ----
            for d in DIRS:
                bank = gb[(d, (s // 16) % 2)]
                if s == 0:
                    rhs = zero8_ap()
                else:
                    W(nc.tensor, "dve", h_done[(d, s - 1)])
                    if d == "f":
                        rhs = Xout[:, (s - 1) * 8: s * 8]
                    else:
                        rhs = ap_of(Xout, NTOK + (T - s) * 8,
                                    [[pstride(Xout), 128], [1, 8]])
                last = None
                for g in range(4):
                    last = nc.tensor.matmul(
                        bank[:, base + 8 * g: base + 8 * g + 8],
                        whh_slice(l, d, g), rhs,
                        start=False, stop=True, skip_group_check=True)
                mm_done[(d, s)] = inc(last, "mm")
            # ---- PE: pregate pieces for chunk c+1 (safe: the target bank's
            # last sigma read was step 16c-1, ordered before via h-dep) ----
            j = s % 16
            if j == 0:
                pregate_bias(l, "f", s // 16 + 1)
            elif j == 1:
                pregate_part(l, "f", s // 16 + 1, 0)
            elif j == 2:
                pregate_part(l, "f", s // 16 + 1, 1)
            elif j == 3:
                pregate_bias(l, "b", s // 16 + 1)
            elif j == 4:
                pregate_part(l, "b", s // 16 + 1, 0)
            elif j == 5:
                pregate_part(l, "b", s // 16 + 1, 1)
            # ---- ACT: sigma over gates (both dirs) ----
            for d in DIRS:
                bank = gb[(d, (s // 16) % 2)]
                W(nc.scalar, "mm", mm_done[(d, s)])
                ins = nc.scalar.activation(u[(d, par)][:, 0:32],
                                           bank[:, base:base + 32], ActFn.Sigmoid)
                sg_done[(d, s)] = inc(ins, "act")
            # ---- DVE: [PQf, spacer, Cf, PQb, spacer, Cb] ----
            # The spacer (a) satisfies the DVE gap-0 RAW hazard between PQ and
            # C and (b) keeps Cf ahead of PQb's semaphore wait in the queue so
            # the fwd chain does not detour through the bwd gates.
            for d in DIRS:
                W(nc.vector, "act", sg_done[(d, s)])
                in0 = ap_of(u[(d, par)], 16, [[pstride(u[(d, par)]), 128], [16, 2], [1, 8]])
                in1 = ap_of(u[(d, par)], 0, [[pstride(u[(d, par)]), 128], [8, 2], [1, 8]])
                out = ap_of(pq[d], 0, [[pstride(pq[d]), 128], [8, 2], [1, 8]])
                nc.vector.scalar_tensor_tensor(out=out, in0=in0, scalar=0.5,
                                               in1=in1, op0=AluOp.subtract,
                                               op1=AluOp.mult)
                nc.vector.memset(dummy[:, :], 0.0)
                ins = nc.vector.scalar_tensor_tensor(
                    out=u[(d, 1 - par)][:, 32:40], in0=pq[d][:, 0:8], scalar=0.5,
                    in1=pq[d][:, 8:16], op0=AluOp.add, op1=AluOp.add)
                c_done[(d, s)] = inc(ins, "dve")
            # ---- ACT: v^ = sigma(4C-2) ----
            for d in DIRS:
                W(nc.scalar, "dve", c_done[(d, s)])
                ins = nc.scalar.activation(vhat[d][:, :], u[(d, 1 - par)][:, 32:40],
                                           ActFn.Sigmoid, scale=4.0, bias=-2.0)
                sc_done[(d, s)] = inc(ins, "act")
            # ---- DVE: h~ = (v^-0.5)*o~ ----
            for d in DIRS:
                W(nc.vector, "act", sc_done[(d, s)])
                if d == "f":
                    dst = Xout[:, s * 8:(s + 1) * 8]
                else:
                    dst = ap_of(Xout, NTOK + (T - 1 - s) * 8,
                                [[pstride(Xout), 128], [1, 8]])
                ins = nc.vector.scalar_tensor_tensor(
                    out=dst, in0=vhat[d][:, :], scalar=0.5,
                    in1=u[(d, par)][:, 24:32], op0=AluOp.subtract, op1=AluOp.mult)
                h_done[(d, s)] = inc(ins, "dve")

    layer(0, XL[1])
    layer(1, XL[2])

    # ---------------- FC (+ per-pair output DMA overlap) ----------------
    # y = 2*Wfc @ [X2f; X2b] (+bias): two bf16 matmuls per 512-col chunk; the
    # per-row bias rides the PSUM->SBUF copy as an ACT Identity bias AP.
    W(nc.tensor, "act", cnt["act"])
    W(nc.tensor, "dve", cnt["dve"])
    wfca_ap = ap_of(fpack, 0, [[pstride(fpack), 128], [1, 64]])
    wfcb_ap = ap_of(fpack, 64, [[pstride(fpack), 128], [1, 64]])
    bias_ap = ap_of(bfc64, 0, [[pstride(bfc64), 64], [1, 1]])
    fc_copy = {}
    n_out = 0
    fc_starts = list(range(0, NTOK, 512))
    for i, st in enumerate(fc_starts):
        w = min(512, NTOK - st)
        bank = tbank[i % 4]
        if i >= 4:
            eng, c0 = fc_copy[i - 4]
            W(nc.tensor, eng, c0)
        nc.tensor.matmul(bank[0:64, 0:w], wfca_ap, XL[2][:, st:st + w],
                         start=True, stop=False, skip_group_check=True)
        ins = nc.tensor.matmul(bank[0:64, 0:w], wfcb_ap,
                               ap_of(XL[2], NTOK + st, [[pstride(XL[2]), 128], [1, w]]),
                               start=False, stop=True, skip_group_check=True)
        mmc = inc(ins, "mm")
        W(nc.scalar, "mm", mmc)
        ins = nc.scalar.activation(y_s[:, st:st + w], bank[0:64, 0:w],
                                   ActFn.Identity, bias=bias_ap)
        fc_copy[i] = ("act", inc(ins, "act"))
        if i % 2 == 1:
            # DMA out the finished 1024-col slice while later chunks compute
            nc.sync.wait_ge(s_act, fc_copy[i][1])
            nc.sync.dma_start(out=y_d[:, st - 512:st + w],
                              in_=y_s[:, st - 512:st + w]).then_inc(s_out, 16)
            n_out += 1
    if len(fc_starts) % 2 == 1:
        st = fc_starts[-1]
        w = min(512, NTOK - st)
        nc.sync.wait_ge(s_act, cnt["act"])
        nc.sync.dma_start(out=y_d[:, st:st + w],
                          in_=y_s[:, st:st + w]).then_inc(s_out, 16)
        n_out += 1

    nc.sync.wait_ge(s_out, 16 * n_out)
    return nc


# ====================== host-side prep & entry point ======================

def _to_bf(a):
    return np.asarray(a, dtype=np.float32).astype(BF)


def prep_weights(inputs):
    """Pre-scaled lhsT tensors per the v2 formulation."""
    out = {}
    for l in (0, 1):
        xin_scale = 1.0 if l == 0 else 2.0
        for dname, suf in (("f", ""), ("b", "r")):
            wih = np.asarray(inputs[f"w_ih_l{l}{suf}"], np.float32)   # [512, Din]
            whh = np.asarray(inputs[f"w_hh_l{l}{suf}"], np.float32)   # [512, 128]
            bsum = (np.asarray(inputs[f"b_ih_l{l}{suf}"], np.float32)
                    + np.asarray(inputs[f"b_hh_l{l}{suf}"], np.float32))
            blk_ih, blk_hh, b4 = [], [], np.zeros((4, 128), np.float32)
            for gi, G in enumerate(GO):
                rows = slice(PT[G] * 128, (PT[G] + 1) * 128)
                sG = 2.0 if G == "g" else 1.0
                blk_ih.append((sG * xin_scale * wih[rows]).T)   # [Din,128]
                blk_hh.append((sG * 2.0 * whh[rows]).T)         # [128,128]
                b4[gi] = sG * bsum[rows]
            wih_cat = np.concatenate(blk_ih, axis=1)            # [Din, 512]
            out[f"whh{l}{dname}"] = _to_bf(np.concatenate(blk_hh, axis=1))
            out[f"b4_{l}{dname}"] = _to_bf(b4)
            if l == 0:
                out[f"wih0{dname}"] = _to_bf(wih_cat)           # [64, 512]
            else:
                out[f"wih1a{dname}"] = _to_bf(wih_cat[0:128])
                out[f"wih1b{dname}"] = _to_bf(wih_cat[128:256])
    wfc = 2.0 * np.asarray(inputs["w_fc"], np.float32).T        # [256, 64]
    out["wfca"] = _to_bf(wfc[0:128])
    out["wfcb"] = _to_bf(wfc[128:256])
    out["bfc"] = np.asarray(inputs["b_fc"], np.float32).reshape(1, 64)
    return out


def _mask4_np():
    m = np.zeros((4, 512), np.float32)
    for g in range(4):
        for r in range(16):
            m[g, r * 32 + g * 8: r * 32 + g * 8 + 8] = 1.0
    return m.astype(BF)


_NC_CACHE = {}


def _get_nc(T):
    if T not in _NC_CACHE:
        _NC_CACHE[T] = build_nc(T)
    return _NC_CACHE[T]


def run_cores(inputs, T=512, n_cores=8, trace=False):
    x = np.asarray(inputs["x"], np.float32)
    per = 8
    wp = prep_weights(inputs)
    wpack = np.concatenate([wp["whh0f"], wp["whh0b"], wp["whh1f"], wp["whh1b"],
                            wp["wih1af"], wp["wih1ab"], wp["wih1bf"], wp["wih1bb"]],
                           axis=1)                                    # [128, 4096]
    wih0p = np.concatenate([wp["wih0f"], wp["wih0b"]], axis=1)        # [64, 1024]
    bpack = np.concatenate([wp["b4_0f"], wp["b4_0b"], wp["b4_1f"], wp["b4_1b"],
                            _mask4_np()], axis=1)                     # [4, 1024]
    fpack = np.concatenate([wp["wfca"], wp["wfcb"],
                            np.zeros((128, 8), BF)], axis=1)          # [128, 136]
    common = {
        "wpack": wpack, "wih0p": wih0p, "bpack": bpack, "fpack": fpack,
        "bfc64": wp["bfc"].reshape(64, 1),
    }
    in_maps = []
    for c in range(n_cores):
        m = dict(common)
        xc = x[c * per:(c + 1) * per, :T]                     # [8, T, 64]
        m["x0"] = np.ascontiguousarray(
            xc.transpose(2, 1, 0).reshape(64, T * 8)).astype(BF)
        in_maps.append(m)

    nc = _get_nc(T)
    res = run_bass_kernel_spmd(nc, in_maps, core_ids=list(range(n_cores)), trace=trace)
    outs = []
    for c in range(n_cores):
        yc = res.results[c]["y"]
        outs.append(yc.reshape(64, T, 8).transpose(2, 1, 0))
    return np.concatenate(outs, axis=0), res


def kernel(**inputs):
    y, _ = run_cores(inputs, T=512, n_cores=8)
    return y.astype(np.float32)
